# revision 12
# baseline (speedup 1.0000x reference)
"""Trainium2 Bass kernel for DeepseekAttention (T=4096, H=2048, 16 heads, d=128).

Tensor-parallel over heads: 8 NeuronCores x 2 heads each (SPMD).

v2 design (fp8 DoubleRow + unit-interleaved schedule):
  - QKV and w_o projections as split-precision fp8 (hi=e4m3 + lo=e5m2,
    3 cross terms) in DoubleRow perf mode: 2 k-tiles contracted per
    instruction at 0.5 cycles/row -> 0.75x the fp16 PE cost, ~0.2% error.
    Weights pre-scaled by 32 on host (e4m3 range); undone via the exp scale
    (1024x on scores) and a final host-side 1/256.
  - Attention S/PV matmuls stay fp16.
  - Softmax denominator: exp tiles accumulated on DVE into 3 fp16
    accumulators, partition-reduced on gpsimd -- no PE work, no PSUM bank.
  - Attention output normalized + split to (e4m3, e5m2) on the fly.
  - Single interleaved schedule: per stage, attention kt-steps for chunk
    c-1, QKV chain segments for chunk c, and w_o PSUM groups for chunk c-2
    are emitted round-robin, so the PE always has independent matmuls to
    hide exp latency and PSUM-evict latency. hid DMA layout is
    chunk-contiguous; constants load on 3 parallel queues.
"""

import numpy as np
import ml_dtypes

import concourse.tile as tile
from concourse import bacc, bass_isa, mybir
from concourse.bass_utils import run_bass_kernel_spmd

T = 4096
HID = 2048
NHEADS = 16
HD = 128
NCORES = 8
HPC = NHEADS // NCORES        # 2 heads per core
FEAT = HPC * HD               # 256 per-core attention features
QKVF = 3 * FEAT               # 768 per-core qkv features
CH = 512                      # T-chunk width
NCH = T // CH                 # 8 chunks
KT = HID // 128               # 16 hidden k-tiles
NKP = KT // 2                 # 8 DoubleRow k-pairs
SCALE = float(HD) ** -0.5
WSCALE = 32.0                 # host pre-scale on w_qkv and w_o (e4m3 range)
ATSCALE = 8.0                 # scale on normalized attention output
MASK_NEG = -3.0e7             # scores carry a 1024x factor; must still kill exp

F16 = mybir.dt.float16
F32 = mybir.dt.float32
E4 = mybir.dt.float8e4
E5 = mybir.dt.float8e5
NE4 = ml_dtypes.float8_e4m3
NE5 = ml_dtypes.float8_e5m2
DR = mybir.MatmulPerfMode.DoubleRow
MUL = mybir.AluOpType.mult


def _build_bass():
    nc = bacc.Bacc("TRN2", target_bir_lowering=False, debug=False,
                   num_devices=NCORES)

    # hid_*: chunk-contiguous layout [128, NCH, KT, CH] so each chunk loads
    # as one 8KB-per-partition contiguous descriptor.
    hid_hi = nc.dram_tensor("hid_hi", [128, NCH, KT, CH], E4, kind="ExternalInput").ap()
    hid_lo = nc.dram_tensor("hid_lo", [128, NCH, KT, CH], E5, kind="ExternalInput").ap()
    wq_hi = nc.dram_tensor("wq_hi", [128, KT, QKVF], E4, kind="ExternalInput").ap()
    wq_lo = nc.dram_tensor("wq_lo", [128, KT, QKVF], E5, kind="ExternalInput").ap()
    wo_hi = nc.dram_tensor("wo_hi", [128, HPC, HID], E4, kind="ExternalInput").ap()
    wo_lo = nc.dram_tensor("wo_lo", [128, HPC, HID], E5, kind="ExternalInput").ap()
    cos2 = nc.dram_tensor("cos2", [128, T], F16, kind="ExternalInput").ap()
    sin2 = nc.dram_tensor("sin2", [128, T], F16, kind="ExternalInput").ap()
    masks = nc.dram_tensor("masks", [128, 4 * CH], F32, kind="ExternalInput").ap()
    out = nc.dram_tensor("out", [T, HID], F16, kind="ExternalOutput").ap()

    with tile.TileContext(nc) as tc:
        _emit(tc, hid_hi, hid_lo, wq_hi, wq_lo, wo_hi, wo_lo, cos2, sin2,
              masks, out)
    nc.compile()
    return nc


def _emit(tc, hid_hi, hid_lo, wq_hi, wq_lo, wo_hi, wo_lo, cos2, sin2,
          masks, out):
    nc = tc.nc
    from contextlib import ExitStack
    ctx = ExitStack()
    with ctx:
        const = ctx.enter_context(tc.tile_pool(name="const", bufs=1))
        hidp = ctx.enter_context(tc.tile_pool(name="hidp", bufs=2))
        ropep = ctx.enter_context(tc.tile_pool(name="ropep", bufs=4))
        persist = ctx.enter_context(tc.tile_pool(name="persist", bufs=1))
        ptp = ctx.enter_context(tc.tile_pool(name="ptp", bufs=8))
        accp = ctx.enter_context(tc.tile_pool(name="accp", bufs=6))
        nrmp = ctx.enter_context(tc.tile_pool(name="nrmp", bufs=2))
        stgp = ctx.enter_context(tc.tile_pool(name="stgp", bufs=2))
        # PSUM: 8 banks: qkv 2, S 2, po 2, wo 2
        psqkv = ctx.enter_context(tc.tile_pool(name="psqkv", bufs=2, space="PSUM"))
        pss = ctx.enter_context(tc.tile_pool(name="pss", bufs=2, space="PSUM"))
        pso = ctx.enter_context(tc.tile_pool(name="pso", bufs=2, space="PSUM"))
        pswo = ctx.enter_context(tc.tile_pool(name="pswo", bufs=2, space="PSUM"))

        # ---- constants on 3 parallel DMA queues ----
        wqh = const.tile([128, KT, QKVF], E4, tag="wqh")
        wql = const.tile([128, KT, QKVF], E5, tag="wql")
        nc.scalar.dma_start(wqh[:], wq_hi[:])
        nc.sync.dma_start(wql[:], wq_lo[:])
        woh = const.tile([128, HPC, HID], E4, tag="woh")
        wol = const.tile([128, HPC, HID], E5, tag="wol")
        cos_sb = const.tile([128, T], F16, tag="cos_sb")
        sin_sb = const.tile([128, T], F16, tag="sin_sb")
        mask_sb = const.tile([128, 4 * CH], F32, tag="mask_sb")
        nc.sync.dma_start(cos_sb[:], cos2[:])
        nc.sync.dma_start(sin_sb[:], sin2[:])

        def load_late_consts():
            nc.gpsimd.dma_start(woh[:], wo_hi[:])
            nc.gpsimd.dma_start(wol[:], wo_lo[:])
            nc.gpsimd.dma_start(mask_sb[:], masks[:])

        # ---- persistent activation tiles ----
        QTR = [[persist.tile([128, CH], F16, tag=f"qtr{h}_{c}", name=f"qtr{h}_{c}")
                for c in range(NCH)] for h in range(HPC)]
        KTR = [[persist.tile([128, CH], F16, tag=f"ktr{h}_{c}", name=f"ktr{h}_{c}")
                for c in range(NCH)] for h in range(HPC)]
        VV = persist.tile([128, HPC * T], F16, tag="vv", name="vv")
        ATH = [persist.tile([128, HPC, CH], E4, tag=f"ath{c}", name=f"ath{c}")
               for c in range(NCH)]
        ATL = [persist.tile([128, HPC, CH], E5, tag=f"atl{c}", name=f"atl{c}")
               for c in range(NCH)]

        hid_tiles = {}

        def load_hid(c):
            hh = hidp.tile([128, KT, CH], E4, tag="hh", name=f"hh{c}")
            hl = hidp.tile([128, KT, CH], E5, tag="hl", name=f"hl{c}")
            nc.gpsimd.dma_start(hh[:], hid_hi[:, c, :, :])
            nc.gpsimd.dma_start(hl[:], hid_lo[:, c, :, :])
            hid_tiles[c] = (hh, hl)

        deferred_rope = []

        def rope_close(ps, ft, c):
            raw = ropep.tile([128, CH], F16, tag="raw", name=f"raw{c}_{ft}")
            nc.scalar.copy(raw[:], ps[:])
            rot = ropep.tile([128, CH], F16, tag="rot", name=f"rot{c}_{ft}")
            nc.sync.dma_start(rot[0:64, :], raw[64:128, :])
            nc.sync.dma_start(rot[64:128, :], raw[0:64, :])
            deferred_rope.append((raw, rot, ft, c))

        def flush_rope():
            for raw, rot, ft, c in deferred_rope:
                h = ft % 2
                ta = ropep.tile([128, CH], F16, tag="ta", name=f"ta{c}_{ft}")
                tb = ropep.tile([128, CH], F16, tag="tb", name=f"tb{c}_{ft}")
                csl = slice(c * CH, (c + 1) * CH)
                nc.vector.tensor_mul(ta[:], raw[:], cos_sb[:, csl])
                nc.vector.tensor_mul(tb[:], rot[:], sin_sb[:, csl])
                dst = QTR[h][c] if ft < 2 else KTR[h][c]
                nc.vector.tensor_add(dst[:], ta[:], tb[:])
            deferred_rope.clear()

        # ------- QKV chain units: 3 fp8-split passes over one chain -------
        PASSES = ((0, 0), (0, 1), (1, 0))   # (w lo?, hid lo?)

        def qkv_units(c):
            """Yield (closure, is_last_of_chain) for chunk c's 8 chains in
            pair-interleaved order (2 chains in flight on 2 PSUM banks)."""
            hh, hl = hid_tiles[c]
            state = {}

            def qk_seg(ft, p):
                def run():
                    if (ft, 'ps') not in state:
                        state[(ft, 'ps')] = psqkv.tile(
                            [128, CH], F32, tag="qkv", name=f"psq{c}_{ft}")
                    ps = state[(ft, 'ps')]
                    wt = wql if PASSES[p][0] else wqh
                    xt = hl if PASSES[p][1] else hh
                    fsl = slice(ft * 128, (ft + 1) * 128)
                    for kp in range(NKP):
                        ksl = slice(2 * kp, 2 * kp + 2)
                        nc.tensor.matmul(ps[:], wt[:, ksl, fsl], xt[:, ksl, :],
                                         start=(p == 0 and kp == 0),
                                         stop=(p == 2 and kp == NKP - 1),
                                         perf_mode=DR)
                    if p == 2:
                        rope_close(ps, ft, c)
                return run

            def v_seg(j, p):
                def run():
                    if (4 + j, 'ps') not in state:
                        state[(4 + j, 'ps')] = psqkv.tile(
                            [128, CH], F32, tag="qkv", name=f"psv{c}_{j}")
                    ps = state[(4 + j, 'ps')]
                    wt = wql if PASSES[p][0] else wqh
                    xt = hl if PASSES[p][1] else hh
                    jsl = slice(j * 128, (j + 1) * 128)
                    vsl = slice(2 * FEAT, 3 * FEAT)
                    for kp in range(NKP):
                        ksl = slice(2 * kp, 2 * kp + 2)
                        nc.tensor.matmul(ps[:, :FEAT],
                                         xt[:, ksl, jsl], wt[:, ksl, vsl],
                                         start=(p == 0 and kp == 0),
                                         stop=(p == 2 and kp == NKP - 1),
                                         perf_mode=DR)
                    if p == 2:
                        kt_ = 4 * c + j
                        nc.scalar.copy(VV[:, kt_ * FEAT:(kt_ + 1) * FEAT],
                                       ps[:, :FEAT])
                return run

            # chain pairs: (q0,k0), (v0,v1), (v2,v3), (q1,k1)
            pairs = [(qk_seg, 0, qk_seg, 2), (v_seg, 0, v_seg, 1),
                     (v_seg, 2, v_seg, 3), (qk_seg, 1, qk_seg, 3)]
            for fa, ia, fb, ib in pairs:
                for p in range(3):
                    yield fa(ia, p)
                    yield fb(ib, p)

        # ------------- attention units for one (chunk, head) -------------
        def attn_units(c, h):
            nkt = 4 * (c + 1)
            st = {}

            def setup():
                st['po'] = pso.tile([128, CH], F32, tag="o", name=f"po{h}_{c}")
                st['acc'] = [accp.tile([128, CH], F16, tag="acc",
                                       name=f"acc{h}_{c}_{i}") for i in range(3)]
                st['first'] = [c > 0, c > 0, c > 0]
                if c == 0:
                    for a in st['acc']:
                        nc.any.memset(a[:], 0.0)
                st['pending'] = []

            def s_exp(kt):
                r = kt - 4 * c
                qo = 128 * r if r > 0 else 0
                ps = pss.tile([128, CH], F32, tag="s", name=f"ps{h}_{c}_{kt}")
                nc.tensor.matmul(
                    ps[:, qo:],
                    KTR[h][kt // 4][:, (kt % 4) * 128:(kt % 4 + 1) * 128],
                    QTR[h][c][:, qo:],
                    start=True, stop=True)
                if r >= 0:
                    nc.vector.tensor_add(
                        ps[:, qo:qo + 128], ps[:, qo:qo + 128],
                        mask_sb[:, r * CH + qo:r * CH + qo + 128])
                pt = ptp.tile([128, CH], F16, tag="pt", name=f"pt{h}_{c}_{kt}")
                nc.scalar.activation(pt[:, qo:], ps[:, qo:],
                                     mybir.ActivationFunctionType.Exp,
                                     scale=SCALE / (WSCALE * WSCALE))
                return kt, qo, pt

            def pv_acc(pend):
                kt, qo, pt = pend
                nc.tensor.matmul(
                    st['po'][:, qo:],
                    VV[:, kt * FEAT + h * 128:kt * FEAT + (h + 1) * 128],
                    pt[:, qo:],
                    start=(kt == 0), stop=(kt == nkt - 1))
                i = kt % 3
                if st['first'][i]:
                    nc.vector.tensor_copy(st['acc'][i][:], pt[:])
                    st['first'][i] = False
                else:
                    nc.vector.tensor_add(st['acc'][i][:, qo:],
                                         st['acc'][i][:, qo:], pt[:, qo:])

            def unit(kt):
                def run():
                    if kt == 0:
                        setup()
                    st['pending'].append(s_exp(kt))
                    if len(st['pending']) > 2:
                        pv_acc(st['pending'].pop(0))
                return run

            def finish():
                for pend in st['pending']:
                    pv_acc(pend)
                st['pending'] = []
                acc = st['acc']
                nc.vector.tensor_add(acc[0][:], acc[0][:], acc[1][:])
                nc.vector.tensor_add(acc[0][:], acc[0][:], acc[2][:])
                pdall = nrmp.tile([128, CH], F32, tag="pdall", name=f"pd{h}_{c}")
                nc.gpsimd.partition_all_reduce(
                    pdall[:], acc[0][:], channels=128,
                    reduce_op=bass_isa.ReduceOp.add)
                binv = nrmp.tile([128, CH], F32, tag="binv", name=f"bi{h}_{c}")
                nc.vector.reciprocal(binv[:], pdall[:])
                at16 = nrmp.tile([128, CH], F16, tag="at16", name=f"a16{h}_{c}")
                nc.vector.scalar_tensor_tensor(
                    at16[:], st['po'][:], ATSCALE / WSCALE, binv[:],
                    op0=MUL, op1=MUL)
                nc.vector.tensor_copy(ATH[c][:, h, :], at16[:])
                nc.vector.tensor_sub(ATL[c][:, h, :], at16[:], ATH[c][:, h, :])

            return [unit(kt) for kt in range(nkt)], finish

        # --------- w_o units (fp8 split DoubleRow, head-paired) ----------
        def wo_units(c, split_last_dma=False):
            stt = {}

            def unit(j, n):
                def run():
                    tt = 4 * c + j
                    if n == 0:
                        stt[j] = stgp.tile([128, HID], F16, tag="stg",
                                           name=f"stg{tt}")
                    stg = stt[j]
                    jsl = slice(j * 128, (j + 1) * 128)
                    nsl = slice(n * CH, (n + 1) * CH)
                    pw = pswo.tile([128, CH], F32, tag="w", name=f"pw{tt}_{n}")
                    for i, (a, w) in enumerate(
                            ((ATH[c], woh), (ATH[c], wol), (ATL[c], woh))):
                        nc.tensor.matmul(pw[:], a[:, :, jsl], w[:, :, nsl],
                                         start=(i == 0), stop=(i == 2),
                                         perf_mode=DR)
                    if (n + tt) % 4 < 2:
                        nc.vector.tensor_copy(stg[:, nsl], pw[:])
                    else:
                        nc.scalar.copy(stg[:, nsl], pw[:])
                    if split_last_dma:
                        if n == 1:
                            eng = nc.sync if tt % 2 == 0 else nc.scalar
                            eng.dma_start(out[tt * 128:(tt + 1) * 128, :HID // 2],
                                          stg[:, :HID // 2])
                        elif n == 3:
                            eng = nc.scalar if tt % 2 == 0 else nc.gpsimd
                            eng.dma_start(out[tt * 128:(tt + 1) * 128, HID // 2:],
                                          stg[:, HID // 2:])
                    elif n == 3:
                        eng = nc.sync if tt % 2 == 0 else nc.scalar
                        eng.dma_start(out[tt * 128:(tt + 1) * 128, :], stg[:])
                return run

            return [unit(j, n) for j in range(4) for n in range(4)]

        def weave(primary, fillers):
            """Emit primary units with fillers spread evenly between them."""
            np_, nf = len(primary), len(fillers)
            fi = 0
            for i, u in enumerate(primary):
                u()
                want = (i + 1) * nf // np_
                while fi < want:
                    fillers[fi]()
                    fi += 1
            while fi < nf:
                fillers[fi]()
                fi += 1

        # ================= main schedule =================
        load_hid(0)
        load_hid(1)
        load_late_consts()
        for u in qkv_units(0):
            u()
        flush_rope()

        for k in range(1, NCH + 1):
            c_attn = k - 1
            c_wo = k - 2
            if k < NCH:
                if k + 1 < NCH:
                    load_hid(k + 1)
                h0_units, h0_fin = attn_units(c_attn, 0)
                h1_units, h1_fin = attn_units(c_attn, 1)
                qun = list(qkv_units(k))
                woun = wo_units(c_wo) if c_wo >= 0 else []
                nq = len(qun)
                # h0 attention woven with first half of qkv; h1 with rest + wo
                weave(h0_units, qun[:nq // 2])
                h0_fin()
                weave(h1_units, qun[nq // 2:] + woun)
                h1_fin()
                flush_rope()
            else:
                h0_units, h0_fin = attn_units(c_attn, 0)
                h1_units, h1_fin = attn_units(c_attn, 1)
                woun = wo_units(c_wo)
                weave(h0_units, woun[:8])
                h0_fin()
                weave(h1_units, woun[8:])
                h1_fin()
        for u in wo_units(NCH - 1, split_last_dma=True):
            u()


_NC_CACHE = None


def _get_nc():
    global _NC_CACHE
    if _NC_CACHE is None:
        _NC_CACHE = _build_bass()
    return _NC_CACHE


def _split8(x):
    hi = np.ascontiguousarray(x).astype(NE4)
    lo = (x - hi.astype(np.float32)).astype(NE5)
    return hi, np.ascontiguousarray(lo)


def prepare_inputs(hidden_states, positions, w_qkv, w_o):
    """Host-side sharding/preprocessing -> list of per-core input maps."""
    hidden_states = np.asarray(hidden_states, dtype=np.float32)
    positions = np.asarray(positions)
    w_qkv = np.asarray(w_qkv, dtype=np.float32)
    w_o = np.asarray(w_o, dtype=np.float32)

    # hidden^T -> [128, NCH, KT, CH] chunk-contiguous fp8 hi/lo
    hidT = (hidden_states.T.reshape(KT, 128, NCH, CH)
            .transpose(1, 2, 0, 3))
    hid_hi, hid_lo = _split8(hidT)

    pos = positions.astype(np.float32)
    half = HD // 2
    inv_freq = 1.0 / (10000.0 ** (np.arange(half, dtype=np.float32) / half))
    freqs = np.outer(pos, inv_freq)          # [T, 64]
    cos = np.cos(freqs).T                    # [64, T]
    sin = np.sin(freqs).T
    cos2 = np.concatenate([cos, cos], axis=0).astype(np.float16)
    sin2 = np.concatenate([-sin, sin], axis=0).astype(np.float16)

    k_idx = np.arange(128)[:, None]
    q_idx = np.arange(CH)[None, :]
    mblocks = [np.where(128 * r + k_idx <= q_idx, 0.0, MASK_NEG).astype(np.float32)
               for r in range(4)]
    masks_np = np.concatenate(mblocks, axis=1)

    in_maps = []
    for core in range(NCORES):
        heads = [HPC * core + i for i in range(HPC)]
        wq = [w_qkv[:, h * HD:(h + 1) * HD] for h in heads]
        wk = [w_qkv[:, FEAT * NCORES + h * HD:FEAT * NCORES + (h + 1) * HD]
              for h in heads]
        wv = [w_qkv[:, 2 * FEAT * NCORES + h * HD:2 * FEAT * NCORES + (h + 1) * HD]
              for h in heads]
        wqkv_core = np.concatenate(wq + wk + wv, axis=1) * WSCALE
        wqkv_core = wqkv_core.reshape(KT, 128, QKVF).transpose(1, 0, 2)
        wq_hi, wq_lo = _split8(wqkv_core)
        wo_core = np.stack(
            [w_o[h * HD:(h + 1) * HD, :] for h in heads], axis=0) * WSCALE
        wo_core = wo_core.transpose(1, 0, 2)   # [128, HPC, HID]
        wo_hi, wo_lo = _split8(wo_core)
        in_maps.append({
            "hid_hi": hid_hi,
            "hid_lo": hid_lo,
            "wq_hi": wq_hi,
            "wq_lo": wq_lo,
            "wo_hi": wo_hi,
            "wo_lo": wo_lo,
            "cos2": cos2,
            "sin2": sin2,
            "masks": masks_np,
        })
    return in_maps


def kernel(hidden_states, positions, w_qkv, w_o):
    in_maps = prepare_inputs(hidden_states, positions, w_qkv, w_o)
    nc = _get_nc()
    try:
        res = run_bass_kernel_spmd(nc, in_maps, core_ids=list(range(NCORES)))
    except Exception:
        # transient device wedge from a prior crashed process: retry once
        res = run_bass_kernel_spmd(nc, in_maps, core_ids=list(range(NCORES)))
    acc = res.results[0]["out"].astype(np.float32)
    for i in range(1, NCORES):
        acc += res.results[i]["out"].astype(np.float32)
    return acc * (1.0 / (ATSCALE * WSCALE))


# revision 19
# speedup vs baseline: 1.0230x; 1.0230x over previous
"""Trainium2 Bass kernel for DeepseekAttention (T=4096, H=2048, 16 heads, d=128).

Tensor-parallel over heads: 8 NeuronCores x 2 heads each (SPMD).

v2 design (fp8 DoubleRow + unit-interleaved schedule):
  - QKV and w_o projections as split-precision fp8 (hi=e4m3 + lo=e5m2,
    3 cross terms) in DoubleRow perf mode: 2 k-tiles contracted per
    instruction at 0.5 cycles/row -> 0.75x the fp16 PE cost, ~0.2% error.
    Weights pre-scaled by 32 on host (e4m3 range); undone via the exp scale
    (1024x on scores) and a final host-side 1/256.
  - Attention S/PV matmuls stay fp16.
  - Softmax denominator: exp tiles accumulated on DVE into 3 fp16
    accumulators, partition-reduced on gpsimd -- no PE work, no PSUM bank.
  - Attention output normalized + split to (e4m3, e5m2) on the fly.
  - Single interleaved schedule: per stage, attention kt-steps for chunk
    c-1, QKV chain segments for chunk c, and w_o PSUM groups for chunk c-2
    are emitted round-robin, so the PE always has independent matmuls to
    hide exp latency and PSUM-evict latency. hid DMA layout is
    chunk-contiguous; constants load on 3 parallel queues.
"""

import numpy as np
import ml_dtypes

import concourse.tile as tile
from concourse import bacc, bass_isa, mybir
from concourse.bass_utils import run_bass_kernel_spmd

T = 4096
HID = 2048
NHEADS = 16
HD = 128
NCORES = 8
HPC = NHEADS // NCORES        # 2 heads per core
FEAT = HPC * HD               # 256 per-core attention features
QKVF = 3 * FEAT               # 768 per-core qkv features
CH = 512                      # T-chunk width
NCH = T // CH                 # 8 chunks
KT = HID // 128               # 16 hidden k-tiles
NKP = KT // 2                 # 8 DoubleRow k-pairs
SCALE = float(HD) ** -0.5
WSCALE = 32.0                 # host pre-scale on w_qkv and w_o (e4m3 range)
ATSCALE = 8.0                 # scale on normalized attention output
MASK_NEG = -3.0e7             # scores carry a 1024x factor; must still kill exp

F16 = mybir.dt.float16
F32 = mybir.dt.float32
E4 = mybir.dt.float8e4
E5 = mybir.dt.float8e5
NE4 = ml_dtypes.float8_e4m3
NE5 = ml_dtypes.float8_e5m2
DR = mybir.MatmulPerfMode.DoubleRow
MUL = mybir.AluOpType.mult


def _build_bass():
    nc = bacc.Bacc("TRN2", target_bir_lowering=False, debug=False,
                   num_devices=NCORES)

    # hid_*: chunk-contiguous layout [128, NCH, KT, CH] so each chunk loads
    # as one 8KB-per-partition contiguous descriptor.
    hid_hi = nc.dram_tensor("hid_hi", [128, NCH, KT, CH], E4, kind="ExternalInput").ap()
    hid_lo = nc.dram_tensor("hid_lo", [128, NCH, KT, CH], E5, kind="ExternalInput").ap()
    wq_hi = nc.dram_tensor("wq_hi", [128, KT, QKVF], E4, kind="ExternalInput").ap()
    wq_lo = nc.dram_tensor("wq_lo", [128, KT, QKVF], E5, kind="ExternalInput").ap()
    wo_hi = nc.dram_tensor("wo_hi", [128, HPC, HID], E4, kind="ExternalInput").ap()
    wo_lo = nc.dram_tensor("wo_lo", [128, HPC, HID], E5, kind="ExternalInput").ap()
    cos2 = nc.dram_tensor("cos2", [128, T], F16, kind="ExternalInput").ap()
    sin2 = nc.dram_tensor("sin2", [128, T], F16, kind="ExternalInput").ap()
    # causal-mask-as-matmul constants: ltri[j,k] = MASK_NEG for j < k,
    # eyepad = [I_128 | 0]; mask block = ltri.T @ eyepad written into PSUM
    # as the opener of each diagonal S accumulation group.
    ltri = nc.dram_tensor("ltri", [128, 128], mybir.dt.bfloat16,
                          kind="ExternalInput").ap()
    eyepad = nc.dram_tensor("eyepad", [128, CH], mybir.dt.bfloat16,
                            kind="ExternalInput").ap()
    out = nc.dram_tensor("out", [T, HID], F16, kind="ExternalOutput").ap()

    with tile.TileContext(nc) as tc:
        _emit(tc, hid_hi, hid_lo, wq_hi, wq_lo, wo_hi, wo_lo, cos2, sin2,
              ltri, eyepad, out)
    nc.compile()
    return nc


def _emit(tc, hid_hi, hid_lo, wq_hi, wq_lo, wo_hi, wo_lo, cos2, sin2,
          ltri, eyepad, out):
    nc = tc.nc
    from contextlib import ExitStack
    ctx = ExitStack()
    with ctx:
        const = ctx.enter_context(tc.tile_pool(name="const", bufs=1))
        hidp = ctx.enter_context(tc.tile_pool(name="hidp", bufs=2))
        ropep = ctx.enter_context(tc.tile_pool(name="ropep", bufs=4))
        persist = ctx.enter_context(tc.tile_pool(name="persist", bufs=1))
        ptp = ctx.enter_context(tc.tile_pool(name="ptp", bufs=8))
        accp = ctx.enter_context(tc.tile_pool(name="accp", bufs=6))
        nrmp = ctx.enter_context(tc.tile_pool(name="nrmp", bufs=2))
        stgp = ctx.enter_context(tc.tile_pool(name="stgp", bufs=2))
        # PSUM: 8 banks: qkv 2, S 2, po 2, wo 2
        psqkv = ctx.enter_context(tc.tile_pool(name="psqkv", bufs=2, space="PSUM"))
        pss = ctx.enter_context(tc.tile_pool(name="pss", bufs=2, space="PSUM"))
        pso = ctx.enter_context(tc.tile_pool(name="pso", bufs=2, space="PSUM"))
        pswo = ctx.enter_context(tc.tile_pool(name="pswo", bufs=2, space="PSUM"))

        # ---- constants; load order tuned for the serial DMA device ----
        wqh = const.tile([128, KT, QKVF], E4, tag="wqh")
        wql = const.tile([128, KT, QKVF], E5, tag="wql")
        woh = const.tile([128, HPC, HID], E4, tag="woh")
        wol = const.tile([128, HPC, HID], E5, tag="wol")
        cos_sb = const.tile([128, T], F16, tag="cos_sb")
        sin_sb = const.tile([128, T], F16, tag="sin_sb")
        ltri_sb = const.tile([128, 128], mybir.dt.bfloat16, tag="ltri_sb")
        eye_sb = const.tile([128, CH], mybir.dt.bfloat16, tag="eye_sb")

        def load_early_consts():
            # scalar queue: wqh (needed first); sync queue: wql (pass 3)
            nc.scalar.dma_start(wqh[:], wq_hi[:])
            nc.sync.dma_start(wql[:], wq_lo[:])

        def load_late_consts():
            nc.gpsimd.dma_start(cos_sb[:], cos2[:])
            nc.gpsimd.dma_start(sin_sb[:], sin2[:])
            nc.scalar.dma_start(ltri_sb[:], ltri[:])
            nc.scalar.dma_start(eye_sb[:], eyepad[:])
            nc.gpsimd.dma_start(woh[:], wo_hi[:])
            nc.gpsimd.dma_start(wol[:], wo_lo[:])

        # ---- persistent activation tiles ----
        QTR = [[persist.tile([128, CH], F16, tag=f"qtr{h}_{c}", name=f"qtr{h}_{c}")
                for c in range(NCH)] for h in range(HPC)]
        KTR = [[persist.tile([128, CH], F16, tag=f"ktr{h}_{c}", name=f"ktr{h}_{c}")
                for c in range(NCH)] for h in range(HPC)]
        VV = persist.tile([128, HPC * T], F16, tag="vv", name="vv")
        ATH = [persist.tile([128, HPC, CH], E4, tag=f"ath{c}", name=f"ath{c}")
               for c in range(NCH)]
        ATL = [persist.tile([128, HPC, CH], E5, tag=f"atl{c}", name=f"atl{c}")
               for c in range(NCH)]

        hid_tiles = {}

        def load_hid(c):
            hh = hidp.tile([128, KT, CH], E4, tag="hh", name=f"hh{c}")
            hl = hidp.tile([128, KT, CH], E5, tag="hl", name=f"hl{c}")
            nc.gpsimd.dma_start(hh[:], hid_hi[:, c, :, :])
            nc.gpsimd.dma_start(hl[:], hid_lo[:, c, :, :])
            hid_tiles[c] = (hh, hl)

        deferred_rope = []

        def rope_close(ps, ft, c):
            raw = ropep.tile([128, CH], F16, tag="raw", name=f"raw{c}_{ft}")
            nc.scalar.copy(raw[:], ps[:])
            rot = ropep.tile([128, CH], F16, tag="rot", name=f"rot{c}_{ft}")
            nc.sync.dma_start(rot[0:64, :], raw[64:128, :])
            nc.sync.dma_start(rot[64:128, :], raw[0:64, :])
            deferred_rope.append((raw, rot, ft, c))

        def flush_rope():
            for raw, rot, ft, c in deferred_rope:
                h = ft % 2
                ta = ropep.tile([128, CH], F16, tag="ta", name=f"ta{c}_{ft}")
                tb = ropep.tile([128, CH], F16, tag="tb", name=f"tb{c}_{ft}")
                csl = slice(c * CH, (c + 1) * CH)
                nc.vector.tensor_mul(ta[:], raw[:], cos_sb[:, csl])
                nc.vector.tensor_mul(tb[:], rot[:], sin_sb[:, csl])
                dst = QTR[h][c] if ft < 2 else KTR[h][c]
                nc.vector.tensor_add(dst[:], ta[:], tb[:])
            deferred_rope.clear()

        # ------- QKV chain units: 3 fp8-split passes over one chain -------
        PASSES = ((0, 0), (0, 1), (1, 0))   # (w lo?, hid lo?)

        def qkv_units(c):
            """Yield (closure, is_last_of_chain) for chunk c's 8 chains in
            pair-interleaved order (2 chains in flight on 2 PSUM banks)."""
            hh, hl = hid_tiles[c]
            state = {}

            def qk_seg(ft, p):
                def run():
                    if (ft, 'ps') not in state:
                        state[(ft, 'ps')] = psqkv.tile(
                            [128, CH], F32, tag="qkv", name=f"psq{c}_{ft}")
                    ps = state[(ft, 'ps')]
                    wt = wql if PASSES[p][0] else wqh
                    xt = hl if PASSES[p][1] else hh
                    fsl = slice(ft * 128, (ft + 1) * 128)
                    for kp in range(NKP):
                        ksl = slice(2 * kp, 2 * kp + 2)
                        nc.tensor.matmul(ps[:], wt[:, ksl, fsl], xt[:, ksl, :],
                                         start=(p == 0 and kp == 0),
                                         stop=(p == 2 and kp == NKP - 1),
                                         perf_mode=DR)
                    if p == 2:
                        rope_close(ps, ft, c)
                return run

            def v_seg(j, p):
                def run():
                    if (4 + j, 'ps') not in state:
                        state[(4 + j, 'ps')] = psqkv.tile(
                            [128, CH], F32, tag="qkv", name=f"psv{c}_{j}")
                    ps = state[(4 + j, 'ps')]
                    wt = wql if PASSES[p][0] else wqh
                    xt = hl if PASSES[p][1] else hh
                    jsl = slice(j * 128, (j + 1) * 128)
                    vsl = slice(2 * FEAT, 3 * FEAT)
                    for kp in range(NKP):
                        ksl = slice(2 * kp, 2 * kp + 2)
                        nc.tensor.matmul(ps[:, :FEAT],
                                         xt[:, ksl, jsl], wt[:, ksl, vsl],
                                         start=(p == 0 and kp == 0),
                                         stop=(p == 2 and kp == NKP - 1),
                                         perf_mode=DR)
                    if p == 2:
                        kt_ = 4 * c + j
                        nc.scalar.copy(VV[:, kt_ * FEAT:(kt_ + 1) * FEAT],
                                       ps[:, :FEAT])
                return run

            # chain pairs: (q0,k0), (v0,v1), (v2,v3), (q1,k1)
            pairs = [(qk_seg, 0, qk_seg, 2), (v_seg, 0, v_seg, 1),
                     (v_seg, 2, v_seg, 3), (qk_seg, 1, qk_seg, 3)]
            for fa, ia, fb, ib in pairs:
                for p in range(3):
                    yield fa(ia, p)
                    yield fb(ib, p)

        # ------------- attention units for one (chunk, head) -------------
        def attn_units(c, h):
            nkt = 4 * (c + 1)
            st = {}

            def setup():
                st['po'] = pso.tile([128, CH], F32, tag="o", name=f"po{h}_{c}")
                st['acc'] = [accp.tile([128, CH], F16, tag="acc",
                                       name=f"acc{h}_{c}_{i}") for i in range(3)]
                st['first'] = [c > 0, c > 0, c > 0]
                if c == 0:
                    for a in st['acc']:
                        nc.any.memset(a[:], 0.0)
                st['pending'] = []

            def s_exp(kt):
                r = kt - 4 * c
                qo = 128 * r if r > 0 else 0
                ps = pss.tile([128, CH], F32, tag="s", name=f"ps{h}_{c}_{kt}")
                if r >= 0:
                    # open the group with the causal mask: ltri.T @ [I|0]
                    nc.tensor.matmul(ps[:, qo:], ltri_sb[:],
                                     eye_sb[:, :CH - qo],
                                     start=True, stop=False)
                nc.tensor.matmul(
                    ps[:, qo:],
                    KTR[h][kt // 4][:, (kt % 4) * 128:(kt % 4 + 1) * 128],
                    QTR[h][c][:, qo:],
                    start=(r < 0), stop=True)
                pt = ptp.tile([128, CH], F16, tag="pt", name=f"pt{h}_{c}_{kt}")
                nc.scalar.activation(pt[:, qo:], ps[:, qo:],
                                     mybir.ActivationFunctionType.Exp,
                                     scale=SCALE / (WSCALE * WSCALE))
                return kt, qo, pt

            def pv_acc(pend):
                kt, qo, pt = pend
                nc.tensor.matmul(
                    st['po'][:, qo:],
                    VV[:, kt * FEAT + h * 128:kt * FEAT + (h + 1) * 128],
                    pt[:, qo:],
                    start=(kt == 0), stop=(kt == nkt - 1))
                i = kt % 3
                if st['first'][i]:
                    nc.vector.tensor_copy(st['acc'][i][:], pt[:])
                    st['first'][i] = False
                else:
                    nc.vector.tensor_add(st['acc'][i][:, qo:],
                                         st['acc'][i][:, qo:], pt[:, qo:])

            def unit(kt):
                def run():
                    if kt == 0:
                        setup()
                    st['pending'].append(s_exp(kt))
                    if len(st['pending']) > 2:
                        pv_acc(st['pending'].pop(0))
                return run

            def finish():
                for pend in st['pending']:
                    pv_acc(pend)
                st['pending'] = []
                acc = st['acc']
                nc.vector.tensor_add(acc[0][:], acc[0][:], acc[1][:])
                nc.vector.tensor_add(acc[0][:], acc[0][:], acc[2][:])
                pdall = nrmp.tile([128, CH], F32, tag="pdall", name=f"pd{h}_{c}")
                nc.gpsimd.partition_all_reduce(
                    pdall[:], acc[0][:], channels=128,
                    reduce_op=bass_isa.ReduceOp.add)
                binv = nrmp.tile([128, CH], F32, tag="binv", name=f"bi{h}_{c}")
                nc.vector.reciprocal(binv[:], pdall[:])
                at16 = nrmp.tile([128, CH], F16, tag="at16", name=f"a16{h}_{c}")
                nc.vector.scalar_tensor_tensor(
                    at16[:], st['po'][:], ATSCALE / WSCALE, binv[:],
                    op0=MUL, op1=MUL)
                nc.vector.tensor_copy(ATH[c][:, h, :], at16[:])
                nc.vector.tensor_sub(ATL[c][:, h, :], at16[:], ATH[c][:, h, :])

            return [unit(kt) for kt in range(nkt)], finish

        # --------- w_o units (fp8 split DoubleRow, head-paired) ----------
        def wo_units(c, split_last_dma=False):
            stt = {}

            def unit(j, n):
                def run():
                    tt = 4 * c + j
                    if n == 0:
                        stt[j] = stgp.tile([128, HID], F16, tag="stg",
                                           name=f"stg{tt}")
                    stg = stt[j]
                    jsl = slice(j * 128, (j + 1) * 128)
                    nsl = slice(n * CH, (n + 1) * CH)
                    pw = pswo.tile([128, CH], F32, tag="w", name=f"pw{tt}_{n}")
                    for i, (a, w) in enumerate(
                            ((ATH[c], woh), (ATH[c], wol), (ATL[c], woh))):
                        nc.tensor.matmul(pw[:], a[:, :, jsl], w[:, :, nsl],
                                         start=(i == 0), stop=(i == 2),
                                         perf_mode=DR)
                    if (n + tt) % 4 < 2:
                        nc.vector.tensor_copy(stg[:, nsl], pw[:])
                    else:
                        nc.scalar.copy(stg[:, nsl], pw[:])
                    if split_last_dma:
                        if n == 1:
                            eng = nc.sync if tt % 2 == 0 else nc.scalar
                            eng.dma_start(out[tt * 128:(tt + 1) * 128, :HID // 2],
                                          stg[:, :HID // 2])
                        elif n == 3:
                            eng = nc.scalar if tt % 2 == 0 else nc.gpsimd
                            eng.dma_start(out[tt * 128:(tt + 1) * 128, HID // 2:],
                                          stg[:, HID // 2:])
                    elif n == 3:
                        eng = nc.sync if tt % 2 == 0 else nc.scalar
                        eng.dma_start(out[tt * 128:(tt + 1) * 128, :], stg[:])
                return run

            return [unit(j, n) for j in range(4) for n in range(4)]

        def weave(primary, fillers):
            """Emit primary units with fillers spread evenly between them."""
            np_, nf = len(primary), len(fillers)
            fi = 0
            for i, u in enumerate(primary):
                u()
                want = (i + 1) * nf // np_
                while fi < want:
                    fillers[fi]()
                    fi += 1
            while fi < nf:
                fillers[fi]()
                fi += 1

        def mix(a, b):
            """Merge two unit lists, spreading b evenly through a."""
            if not b:
                return list(a)
            if not a:
                return list(b)
            res = []
            bi = 0
            for i, u in enumerate(a):
                res.append(u)
                want = (i + 1) * len(b) // len(a)
                while bi < want:
                    res.append(b[bi])
                    bi += 1
            res.extend(b[bi:])
            return res

        # ================= main schedule =================
        load_hid(0)
        load_early_consts()
        load_hid(1)
        load_late_consts()
        for u in qkv_units(0):
            u()
        flush_rope()

        for k in range(1, NCH + 1):
            c_attn = k - 1
            c_wo = k - 2
            if k < NCH:
                if k + 1 < NCH:
                    load_hid(k + 1)
                h0_units, h0_fin = attn_units(c_attn, 0)
                h1_units, h1_fin = attn_units(c_attn, 1)
                qun = list(qkv_units(k))
                woun = wo_units(c_wo) if c_wo >= 0 else []
                fillers = mix(qun, woun)
                nf = len(fillers)
                weave(h0_units, fillers[:nf // 2])
                h0_fin()
                weave(h1_units, fillers[nf // 2:])
                h1_fin()
                flush_rope()
            else:
                h0_units, h0_fin = attn_units(c_attn, 0)
                h1_units, h1_fin = attn_units(c_attn, 1)
                woun = wo_units(c_wo)
                weave(h0_units, woun[:8])
                h0_fin()
                weave(h1_units, woun[8:])
                h1_fin()
        for u in wo_units(NCH - 1, split_last_dma=True):
            u()


_NC_CACHE = None


def _get_nc():
    global _NC_CACHE
    if _NC_CACHE is None:
        _NC_CACHE = _build_bass()
    return _NC_CACHE


def _split8(x):
    hi = np.ascontiguousarray(x).astype(NE4)
    lo = (x - hi.astype(np.float32)).astype(NE5)
    return hi, np.ascontiguousarray(lo)


def prepare_inputs(hidden_states, positions, w_qkv, w_o):
    """Host-side sharding/preprocessing -> list of per-core input maps."""
    hidden_states = np.asarray(hidden_states, dtype=np.float32)
    positions = np.asarray(positions)
    w_qkv = np.asarray(w_qkv, dtype=np.float32)
    w_o = np.asarray(w_o, dtype=np.float32)

    # hidden^T -> [128, NCH, KT, CH] chunk-contiguous fp8 hi/lo
    hidT = (hidden_states.T.reshape(KT, 128, NCH, CH)
            .transpose(1, 2, 0, 3))
    hid_hi, hid_lo = _split8(hidT)

    pos = positions.astype(np.float32)
    half = HD // 2
    inv_freq = 1.0 / (10000.0 ** (np.arange(half, dtype=np.float32) / half))
    freqs = np.outer(pos, inv_freq)          # [T, 64]
    cos = np.cos(freqs).T                    # [64, T]
    sin = np.sin(freqs).T
    cos2 = np.concatenate([cos, cos], axis=0).astype(np.float16)
    sin2 = np.concatenate([-sin, sin], axis=0).astype(np.float16)

    j_idx = np.arange(128)[:, None]
    k_idx = np.arange(128)[None, :]
    ltri_np = np.where(j_idx < k_idx, MASK_NEG, 0.0).astype(ml_dtypes.bfloat16)
    eye_np = np.zeros((128, CH), dtype=ml_dtypes.bfloat16)
    eye_np[:, :128] = np.eye(128)

    in_maps = []
    for core in range(NCORES):
        heads = [HPC * core + i for i in range(HPC)]
        wq = [w_qkv[:, h * HD:(h + 1) * HD] for h in heads]
        wk = [w_qkv[:, FEAT * NCORES + h * HD:FEAT * NCORES + (h + 1) * HD]
              for h in heads]
        wv = [w_qkv[:, 2 * FEAT * NCORES + h * HD:2 * FEAT * NCORES + (h + 1) * HD]
              for h in heads]
        wqkv_core = np.concatenate(wq + wk + wv, axis=1) * WSCALE
        wqkv_core = wqkv_core.reshape(KT, 128, QKVF).transpose(1, 0, 2)
        wq_hi, wq_lo = _split8(wqkv_core)
        wo_core = np.stack(
            [w_o[h * HD:(h + 1) * HD, :] for h in heads], axis=0) * WSCALE
        wo_core = wo_core.transpose(1, 0, 2)   # [128, HPC, HID]
        wo_hi, wo_lo = _split8(wo_core)
        in_maps.append({
            "hid_hi": hid_hi,
            "hid_lo": hid_lo,
            "wq_hi": wq_hi,
            "wq_lo": wq_lo,
            "wo_hi": wo_hi,
            "wo_lo": wo_lo,
            "cos2": cos2,
            "sin2": sin2,
            "ltri": ltri_np,
            "eyepad": eye_np,
        })
    return in_maps


def kernel(hidden_states, positions, w_qkv, w_o):
    in_maps = prepare_inputs(hidden_states, positions, w_qkv, w_o)
    nc = _get_nc()
    try:
        res = run_bass_kernel_spmd(nc, in_maps, core_ids=list(range(NCORES)))
    except Exception:
        # transient device wedge from a prior crashed process: retry once
        res = run_bass_kernel_spmd(nc, in_maps, core_ids=list(range(NCORES)))
    acc = res.results[0]["out"].astype(np.float32)
    for i in range(1, NCORES):
        acc += res.results[i]["out"].astype(np.float32)
    return acc * (1.0 / (ATSCALE * WSCALE))


# revision 26
# speedup vs baseline: 1.0682x; 1.0441x over previous
"""Trainium2 Bass kernel for DeepseekAttention (T=4096, H=2048, 16 heads, d=128).

Tensor-parallel over heads: 8 NeuronCores x 2 heads each (SPMD).

v2 design (fp8 DoubleRow + unit-interleaved schedule):
  - QKV and w_o projections as split-precision fp8 (hi=e4m3 + lo=e5m2,
    3 cross terms) in DoubleRow perf mode: 2 k-tiles contracted per
    instruction at 0.5 cycles/row -> 0.75x the fp16 PE cost, ~0.2% error.
    Weights pre-scaled by 32 on host (e4m3 range); undone via the exp scale
    (1024x on scores) and a final host-side 1/256.
  - Attention S/PV matmuls stay fp16.
  - Softmax denominator: exp tiles accumulated on DVE into 3 fp16
    accumulators, partition-reduced on gpsimd -- no PE work, no PSUM bank.
  - Attention output normalized + split to (e4m3, e5m2) on the fly.
  - Single interleaved schedule: per stage, attention kt-steps for chunk
    c-1, QKV chain segments for chunk c, and w_o PSUM groups for chunk c-2
    are emitted round-robin, so the PE always has independent matmuls to
    hide exp latency and PSUM-evict latency. hid DMA layout is
    chunk-contiguous; constants load on 3 parallel queues.
"""

import numpy as np
import ml_dtypes

import concourse.tile as tile
from concourse import bacc, bass_isa, mybir
from concourse.bass_utils import run_bass_kernel_spmd

T = 4096
HID = 2048
NHEADS = 16
HD = 128
NCORES = 8
HPC = NHEADS // NCORES        # 2 heads per core
FEAT = HPC * HD               # 256 per-core attention features
QKVF = 3 * FEAT               # 768 per-core qkv features
CH = 512                      # T-chunk width
NCH = T // CH                 # 8 chunks
KT = HID // 128               # 16 hidden k-tiles
NKP = KT // 2                 # 8 DoubleRow k-pairs
SCALE = float(HD) ** -0.5
WSCALE = 32.0                 # host pre-scale on w_qkv and w_o (e4m3 range)
ATSCALE = 8.0                 # scale on normalized attention output
MASK_NEG = -3.0e7             # scores carry a 1024x factor; must still kill exp

F16 = mybir.dt.float16
F32 = mybir.dt.float32
E4 = mybir.dt.float8e4
E5 = mybir.dt.float8e5
NE4 = ml_dtypes.float8_e4m3
NE5 = ml_dtypes.float8_e5m2
DR = mybir.MatmulPerfMode.DoubleRow
MUL = mybir.AluOpType.mult


def _build_bass():
    nc = bacc.Bacc("TRN2", target_bir_lowering=False, debug=False,
                   num_devices=NCORES)

    # hid_*: chunk-contiguous layout [128, NCH, KT, CH] so each chunk loads
    # as one 8KB-per-partition contiguous descriptor.
    hid_hi = nc.dram_tensor("hid_hi", [128, NCH, KT, CH], E4, kind="ExternalInput").ap()
    hid_lo = nc.dram_tensor("hid_lo", [128, NCH, KT, CH], E5, kind="ExternalInput").ap()
    # qkv weights split into A (q0 q1 k0 k1 cols) / B (v cols) so the first
    # chains can start after a smaller DMA on the serial DMA device
    wq_hi_a = nc.dram_tensor("wq_hi_a", [128, KT, 4 * HD], E4, kind="ExternalInput").ap()
    wq_lo_a = nc.dram_tensor("wq_lo_a", [128, KT, 4 * HD], E5, kind="ExternalInput").ap()
    wq_hi_b = nc.dram_tensor("wq_hi_b", [128, KT, FEAT], E4, kind="ExternalInput").ap()
    wq_lo_b = nc.dram_tensor("wq_lo_b", [128, KT, FEAT], E5, kind="ExternalInput").ap()
    wo_hi = nc.dram_tensor("wo_hi", [128, HPC, HID], E4, kind="ExternalInput").ap()
    wo_lo = nc.dram_tensor("wo_lo", [128, HPC, HID], E5, kind="ExternalInput").ap()
    cos2 = nc.dram_tensor("cos2", [128, T], F16, kind="ExternalInput").ap()
    sin2 = nc.dram_tensor("sin2", [128, T], F16, kind="ExternalInput").ap()
    # causal-mask-as-matmul constants: ltri[j,k] = MASK_NEG for j < k,
    # eyepad = [I_128 | 0]; mask block = ltri.T @ eyepad written into PSUM
    # as the opener of each diagonal S accumulation group.
    ltri = nc.dram_tensor("ltri", [128, 128], mybir.dt.bfloat16,
                          kind="ExternalInput").ap()
    eyepad = nc.dram_tensor("eyepad", [128, CH], mybir.dt.bfloat16,
                            kind="ExternalInput").ap()
    out = nc.dram_tensor("out", [T, HID], F16, kind="ExternalOutput").ap()

    with tile.TileContext(nc) as tc:
        _emit(tc, hid_hi, hid_lo, wq_hi_a, wq_lo_a, wq_hi_b, wq_lo_b,
              wo_hi, wo_lo, cos2, sin2, ltri, eyepad, out)
    nc.compile()
    return nc


def _emit(tc, hid_hi, hid_lo, wq_hi_a, wq_lo_a, wq_hi_b, wq_lo_b,
          wo_hi, wo_lo, cos2, sin2, ltri, eyepad, out):
    nc = tc.nc
    from contextlib import ExitStack
    ctx = ExitStack()
    with ctx:
        const = ctx.enter_context(tc.tile_pool(name="const", bufs=1))
        hidp = ctx.enter_context(tc.tile_pool(name="hidp", bufs=2))
        ropep = ctx.enter_context(tc.tile_pool(name="ropep", bufs=4))
        persist = ctx.enter_context(tc.tile_pool(name="persist", bufs=1))
        ptp = ctx.enter_context(tc.tile_pool(name="ptp", bufs=8))
        accp = ctx.enter_context(tc.tile_pool(name="accp", bufs=6))
        nrmp = ctx.enter_context(tc.tile_pool(name="nrmp", bufs=2))
        stgp = ctx.enter_context(tc.tile_pool(name="stgp", bufs=2))
        # PSUM: 8 banks: qkv 2, S 2, po 2, wo 2
        psqkv = ctx.enter_context(tc.tile_pool(name="psqkv", bufs=2, space="PSUM"))
        pss = ctx.enter_context(tc.tile_pool(name="pss", bufs=2, space="PSUM"))
        pso = ctx.enter_context(tc.tile_pool(name="pso", bufs=2, space="PSUM"))
        pswo = ctx.enter_context(tc.tile_pool(name="pswo", bufs=2, space="PSUM"))

        # ---- constants; load order tuned for the serial DMA device ----
        wqh_a = const.tile([128, KT, 4 * HD], E4, tag="wqh_a")
        wql_a = const.tile([128, KT, 4 * HD], E5, tag="wql_a")
        wqh_b = const.tile([128, KT, FEAT], E4, tag="wqh_b")
        wql_b = const.tile([128, KT, FEAT], E5, tag="wql_b")
        woh = const.tile([128, HPC, HID], E4, tag="woh")
        wol = const.tile([128, HPC, HID], E5, tag="wol")
        cos_sb = const.tile([128, T], F16, tag="cos_sb")
        sin_sb = const.tile([128, T], F16, tag="sin_sb")
        ltri_sb = const.tile([128, 128], mybir.dt.bfloat16, tag="ltri_sb")
        eye_sb = const.tile([128, CH], mybir.dt.bfloat16, tag="eye_sb")

        def load_early_consts():
            # order on the (serial) DMA device: wqh_a already issued before
            # hid lo; wql_a next; the V columns (B) arrive later.
            nc.scalar.dma_start(wqh_a[:], wq_hi_a[:])
            nc.sync.dma_start(wql_a[:], wq_lo_a[:])
            nc.scalar.dma_start(wqh_b[:], wq_hi_b[:])
            nc.sync.dma_start(wql_b[:], wq_lo_b[:])

        def load_late_consts():
            nc.gpsimd.dma_start(cos_sb[:], cos2[:])
            nc.gpsimd.dma_start(sin_sb[:], sin2[:])
            nc.scalar.dma_start(ltri_sb[:], ltri[:])
            nc.scalar.dma_start(eye_sb[:], eyepad[:])
            nc.gpsimd.dma_start(woh[:], wo_hi[:])
            nc.gpsimd.dma_start(wol[:], wo_lo[:])

        # ---- persistent activation tiles ----
        QTR = [[persist.tile([128, CH], F16, tag=f"qtr{h}_{c}", name=f"qtr{h}_{c}")
                for c in range(NCH)] for h in range(HPC)]
        KTR = [[persist.tile([128, CH], F16, tag=f"ktr{h}_{c}", name=f"ktr{h}_{c}")
                for c in range(NCH)] for h in range(HPC)]
        VV = persist.tile([128, HPC * T], F16, tag="vv", name="vv")
        ATH = [persist.tile([128, HPC, CH], E4, tag=f"ath{c}", name=f"ath{c}")
               for c in range(NCH)]
        ATL = [persist.tile([128, HPC, CH], E5, tag=f"atl{c}", name=f"atl{c}")
               for c in range(NCH)]

        hid_tiles = {}

        def load_hid(c):
            hh = hidp.tile([128, KT, CH], E4, tag="hh", name=f"hh{c}")
            hl = hidp.tile([128, KT, CH], E5, tag="hl", name=f"hl{c}")
            nc.gpsimd.dma_start(hh[:], hid_hi[:, c, :, :])
            nc.gpsimd.dma_start(hl[:], hid_lo[:, c, :, :])
            hid_tiles[c] = (hh, hl)

        deferred_rope = []

        def rope_close(ps, ft, c):
            raw = ropep.tile([128, CH], F16, tag="raw", name=f"raw{c}_{ft}")
            nc.scalar.copy(raw[:], ps[:])
            rot = ropep.tile([128, CH], F16, tag="rot", name=f"rot{c}_{ft}")
            nc.sync.dma_start(rot[0:64, :], raw[64:128, :])
            nc.sync.dma_start(rot[64:128, :], raw[0:64, :])
            deferred_rope.append((raw, rot, ft, c))

        def flush_rope():
            for raw, rot, ft, c in deferred_rope:
                h = ft % 2
                ta = ropep.tile([128, CH], F16, tag="ta", name=f"ta{c}_{ft}")
                tb = ropep.tile([128, CH], F16, tag="tb", name=f"tb{c}_{ft}")
                csl = slice(c * CH, (c + 1) * CH)
                nc.vector.tensor_mul(ta[:], raw[:], cos_sb[:, csl])
                nc.vector.tensor_mul(tb[:], rot[:], sin_sb[:, csl])
                dst = QTR[h][c] if ft < 2 else KTR[h][c]
                nc.vector.tensor_add(dst[:], ta[:], tb[:])
            deferred_rope.clear()

        # ------- QKV chain units: 3 fp8-split passes over one chain -------
        PASSES = ((0, 0), (0, 1), (1, 0))   # (w lo?, hid lo?)

        def qkv_units(c):
            """Yield (closure, is_last_of_chain) for chunk c's 8 chains in
            pair-interleaved order (2 chains in flight on 2 PSUM banks)."""
            hh, hl = hid_tiles[c]
            state = {}

            def qk_seg(ft, p):
                def run():
                    if (ft, 'ps') not in state:
                        state[(ft, 'ps')] = psqkv.tile(
                            [128, CH], F32, tag="qkv", name=f"psq{c}_{ft}")
                    ps = state[(ft, 'ps')]
                    wt = wql_a if PASSES[p][0] else wqh_a
                    xt = hl if PASSES[p][1] else hh
                    fsl = slice(ft * 128, (ft + 1) * 128)
                    for kp in range(NKP):
                        ksl = slice(2 * kp, 2 * kp + 2)
                        nc.tensor.matmul(ps[:], wt[:, ksl, fsl], xt[:, ksl, :],
                                         start=(p == 0 and kp == 0),
                                         stop=(p == 2 and kp == NKP - 1),
                                         perf_mode=DR)
                    if p == 2:
                        rope_close(ps, ft, c)
                return run

            def v_seg(j, p):
                def run():
                    if (4 + j, 'ps') not in state:
                        state[(4 + j, 'ps')] = psqkv.tile(
                            [128, CH], F32, tag="qkv", name=f"psv{c}_{j}")
                    ps = state[(4 + j, 'ps')]
                    wt = wql_b if PASSES[p][0] else wqh_b
                    xt = hl if PASSES[p][1] else hh
                    jsl = slice(j * 128, (j + 1) * 128)
                    for kp in range(NKP):
                        ksl = slice(2 * kp, 2 * kp + 2)
                        nc.tensor.matmul(ps[:, :FEAT],
                                         xt[:, ksl, jsl], wt[:, ksl, :],
                                         start=(p == 0 and kp == 0),
                                         stop=(p == 2 and kp == NKP - 1),
                                         perf_mode=DR)
                    if p == 2:
                        kt_ = 4 * c + j
                        nc.scalar.copy(VV[:, kt_ * FEAT:(kt_ + 1) * FEAT],
                                       ps[:, :FEAT])
                return run

            # chain pairs: q/k first (only need the A weight columns)
            pairs = [(qk_seg, 0, qk_seg, 2), (qk_seg, 1, qk_seg, 3),
                     (v_seg, 0, v_seg, 1), (v_seg, 2, v_seg, 3)]
            for fa, ia, fb, ib in pairs:
                for p in range(3):
                    yield fa(ia, p)
                    yield fb(ib, p)

        # ------------- attention units for one (chunk, head) -------------
        def attn_units(c, h):
            nkt = 4 * (c + 1)
            st = {}

            def setup():
                st['po'] = pso.tile([128, CH], F32, tag="o", name=f"po{h}_{c}")
                st['acc'] = [accp.tile([128, CH], F16, tag="acc",
                                       name=f"acc{h}_{c}_{i}") for i in range(3)]
                st['first'] = [c > 0, c > 0, c > 0]
                if c == 0:
                    for a in st['acc']:
                        nc.any.memset(a[:], 0.0)
                st['pending'] = []

            def s_exp(kt):
                r = kt - 4 * c
                qo = 128 * r if r > 0 else 0
                ps = pss.tile([128, CH], F32, tag="s", name=f"ps{h}_{c}_{kt}")
                if r >= 0:
                    # open the group with the causal mask: ltri.T @ [I|0]
                    nc.tensor.matmul(ps[:, qo:], ltri_sb[:],
                                     eye_sb[:, :CH - qo],
                                     start=True, stop=False)
                nc.tensor.matmul(
                    ps[:, qo:],
                    KTR[h][kt // 4][:, (kt % 4) * 128:(kt % 4 + 1) * 128],
                    QTR[h][c][:, qo:],
                    start=(r < 0), stop=True)
                pt = ptp.tile([128, CH], F16, tag="pt", name=f"pt{h}_{c}_{kt}")
                nc.scalar.activation(pt[:, qo:], ps[:, qo:],
                                     mybir.ActivationFunctionType.Exp,
                                     scale=SCALE / (WSCALE * WSCALE))
                return kt, qo, pt

            def pv_acc(pend):
                kt, qo, pt = pend
                nc.tensor.matmul(
                    st['po'][:, qo:],
                    VV[:, kt * FEAT + h * 128:kt * FEAT + (h + 1) * 128],
                    pt[:, qo:],
                    start=(kt == 0), stop=(kt == nkt - 1))
                i = kt % 3
                if st['first'][i]:
                    nc.vector.tensor_copy(st['acc'][i][:], pt[:])
                    st['first'][i] = False
                else:
                    nc.vector.tensor_add(st['acc'][i][:, qo:],
                                         st['acc'][i][:, qo:], pt[:, qo:])

            def unit(kt):
                def run():
                    if kt == 0:
                        setup()
                    st['pending'].append(s_exp(kt))
                    if len(st['pending']) > 2:
                        pv_acc(st['pending'].pop(0))
                return run

            def finish():
                for pend in st['pending']:
                    pv_acc(pend)
                st['pending'] = []
                acc = st['acc']
                nc.vector.tensor_add(acc[0][:], acc[0][:], acc[1][:])
                nc.vector.tensor_add(acc[0][:], acc[0][:], acc[2][:])
                pdall = nrmp.tile([128, CH], F32, tag="pdall", name=f"pd{h}_{c}")
                nc.gpsimd.partition_all_reduce(
                    pdall[:], acc[0][:], channels=128,
                    reduce_op=bass_isa.ReduceOp.add)
                binv = nrmp.tile([128, CH], F32, tag="binv", name=f"bi{h}_{c}")
                nc.vector.reciprocal(binv[:], pdall[:])
                at16 = nrmp.tile([128, CH], F16, tag="at16", name=f"a16{h}_{c}")
                nc.vector.scalar_tensor_tensor(
                    at16[:], st['po'][:], ATSCALE / WSCALE, binv[:],
                    op0=MUL, op1=MUL)
                nc.vector.tensor_copy(ATH[c][:, h, :], at16[:])
                nc.vector.tensor_sub(ATL[c][:, h, :], at16[:], ATH[c][:, h, :])

            return [unit(kt) for kt in range(nkt)], finish

        # --------- w_o units (fp8 split DoubleRow, head-paired) ----------
        def wo_units(c, split_last_dma=False):
            stt = {}

            def unit(j, n):
                def run():
                    tt = 4 * c + j
                    if n == 0:
                        stt[j] = stgp.tile([128, HID], F16, tag="stg",
                                           name=f"stg{tt}")
                    stg = stt[j]
                    jsl = slice(j * 128, (j + 1) * 128)
                    nsl = slice(n * CH, (n + 1) * CH)
                    pw = pswo.tile([128, CH], F32, tag="w", name=f"pw{tt}_{n}")
                    for i, (a, w) in enumerate(
                            ((ATH[c], woh), (ATH[c], wol), (ATL[c], woh))):
                        nc.tensor.matmul(pw[:], a[:, :, jsl], w[:, :, nsl],
                                         start=(i == 0), stop=(i == 2),
                                         perf_mode=DR)
                    if (n + tt) % 4 < 2:
                        nc.vector.tensor_copy(stg[:, nsl], pw[:])
                    else:
                        nc.scalar.copy(stg[:, nsl], pw[:])
                    if split_last_dma:
                        if n == 1:
                            eng = nc.sync if tt % 2 == 0 else nc.scalar
                            eng.dma_start(out[tt * 128:(tt + 1) * 128, :HID // 2],
                                          stg[:, :HID // 2])
                        elif n == 3:
                            eng = nc.scalar if tt % 2 == 0 else nc.gpsimd
                            eng.dma_start(out[tt * 128:(tt + 1) * 128, HID // 2:],
                                          stg[:, HID // 2:])
                    elif n == 3:
                        eng = nc.sync if tt % 2 == 0 else nc.scalar
                        eng.dma_start(out[tt * 128:(tt + 1) * 128, :], stg[:])
                return run

            return [unit(j, n) for j in range(4) for n in range(4)]

        def weave(primary, fillers):
            """Emit primary units with fillers spread evenly between them."""
            np_, nf = len(primary), len(fillers)
            fi = 0
            for i, u in enumerate(primary):
                u()
                want = (i + 1) * nf // np_
                while fi < want:
                    fillers[fi]()
                    fi += 1
            while fi < nf:
                fillers[fi]()
                fi += 1

        def mix(a, b):
            """Merge two unit lists, spreading b evenly through a."""
            if not b:
                return list(a)
            if not a:
                return list(b)
            res = []
            bi = 0
            for i, u in enumerate(a):
                res.append(u)
                want = (i + 1) * len(b) // len(a)
                while bi < want:
                    res.append(b[bi])
                    bi += 1
            res.extend(b[bi:])
            return res

        # ================= main schedule =================
        load_hid(0)
        load_early_consts()
        load_hid(1)
        load_late_consts()
        for u in qkv_units(0):
            u()
        flush_rope()

        for k in range(1, NCH + 1):
            c_attn = k - 1
            c_wo = k - 2
            if k < NCH:
                if k + 1 < NCH:
                    load_hid(k + 1)
                h0_units, h0_fin = attn_units(c_attn, 0)
                h1_units, h1_fin = attn_units(c_attn, 1)
                qun = list(qkv_units(k))
                woun = wo_units(c_wo) if c_wo >= 0 else []
                # dense blocks (fine weaving micro-gaps the PE and resets
                # its p-state ramp): attention, qkv, attention, qkv, wo.
                for u in h0_units:
                    u()
                h0_fin()
                for u in qun[:12]:      # (q0,k0), (v0,v1) chain pairs
                    u()
                for u in h1_units:
                    u()
                for u in qun[12:]:      # (v2,v3), (q1,k1) chain pairs
                    u()
                for u in woun:
                    u()
                flush_rope()
            else:
                h0_units, h0_fin = attn_units(c_attn, 0)
                h1_units, h1_fin = attn_units(c_attn, 1)
                woun = wo_units(c_wo)
                weave(h0_units, woun[:8])
                h0_fin()
                weave(h1_units, woun[8:])
                h1_fin()
        for u in wo_units(NCH - 1, split_last_dma=True):
            u()


_NC_CACHE = None


def _get_nc():
    global _NC_CACHE
    if _NC_CACHE is None:
        _NC_CACHE = _build_bass()
    return _NC_CACHE


def _split8(x):
    hi = np.ascontiguousarray(x).astype(NE4)
    lo = (x - hi.astype(np.float32)).astype(NE5)
    return hi, np.ascontiguousarray(lo)


def prepare_inputs(hidden_states, positions, w_qkv, w_o):
    """Host-side sharding/preprocessing -> list of per-core input maps."""
    hidden_states = np.asarray(hidden_states, dtype=np.float32)
    positions = np.asarray(positions)
    w_qkv = np.asarray(w_qkv, dtype=np.float32)
    w_o = np.asarray(w_o, dtype=np.float32)

    # hidden^T -> [128, NCH, KT, CH] chunk-contiguous fp8 hi/lo
    hidT = (hidden_states.T.reshape(KT, 128, NCH, CH)
            .transpose(1, 2, 0, 3))
    hid_hi, hid_lo = _split8(hidT)

    pos = positions.astype(np.float32)
    half = HD // 2
    inv_freq = 1.0 / (10000.0 ** (np.arange(half, dtype=np.float32) / half))
    freqs = np.outer(pos, inv_freq)          # [T, 64]
    cos = np.cos(freqs).T                    # [64, T]
    sin = np.sin(freqs).T
    cos2 = np.concatenate([cos, cos], axis=0).astype(np.float16)
    sin2 = np.concatenate([-sin, sin], axis=0).astype(np.float16)

    j_idx = np.arange(128)[:, None]
    k_idx = np.arange(128)[None, :]
    ltri_np = np.where(j_idx < k_idx, MASK_NEG, 0.0).astype(ml_dtypes.bfloat16)
    eye_np = np.zeros((128, CH), dtype=ml_dtypes.bfloat16)
    eye_np[:, :128] = np.eye(128)

    in_maps = []
    for core in range(NCORES):
        heads = [HPC * core + i for i in range(HPC)]
        wq = [w_qkv[:, h * HD:(h + 1) * HD] for h in heads]
        wk = [w_qkv[:, FEAT * NCORES + h * HD:FEAT * NCORES + (h + 1) * HD]
              for h in heads]
        wv = [w_qkv[:, 2 * FEAT * NCORES + h * HD:2 * FEAT * NCORES + (h + 1) * HD]
              for h in heads]
        wqkv_core = np.concatenate(wq + wk + wv, axis=1) * WSCALE
        wqkv_core = wqkv_core.reshape(KT, 128, QKVF).transpose(1, 0, 2)
        wq_hi, wq_lo = _split8(wqkv_core)
        wq_hi_a = np.ascontiguousarray(wq_hi[:, :, :4 * HD])
        wq_hi_b = np.ascontiguousarray(wq_hi[:, :, 4 * HD:])
        wq_lo_a = np.ascontiguousarray(wq_lo[:, :, :4 * HD])
        wq_lo_b = np.ascontiguousarray(wq_lo[:, :, 4 * HD:])
        wo_core = np.stack(
            [w_o[h * HD:(h + 1) * HD, :] for h in heads], axis=0) * WSCALE
        wo_core = wo_core.transpose(1, 0, 2)   # [128, HPC, HID]
        wo_hi, wo_lo = _split8(wo_core)
        in_maps.append({
            "hid_hi": hid_hi,
            "hid_lo": hid_lo,
            "wq_hi_a": wq_hi_a,
            "wq_lo_a": wq_lo_a,
            "wq_hi_b": wq_hi_b,
            "wq_lo_b": wq_lo_b,
            "wo_hi": wo_hi,
            "wo_lo": wo_lo,
            "cos2": cos2,
            "sin2": sin2,
            "ltri": ltri_np,
            "eyepad": eye_np,
        })
    return in_maps


def kernel(hidden_states, positions, w_qkv, w_o):
    in_maps = prepare_inputs(hidden_states, positions, w_qkv, w_o)
    nc = _get_nc()
    try:
        res = run_bass_kernel_spmd(nc, in_maps, core_ids=list(range(NCORES)))
    except Exception:
        # transient device wedge from a prior crashed process: retry once
        res = run_bass_kernel_spmd(nc, in_maps, core_ids=list(range(NCORES)))
    acc = res.results[0]["out"].astype(np.float32)
    for i in range(1, NCORES):
        acc += res.results[i]["out"].astype(np.float32)
    return acc * (1.0 / (ATSCALE * WSCALE))


# revision 28
# speedup vs baseline: 1.1355x; 1.0630x over previous
"""Trainium2 Bass kernel for DeepseekAttention (T=4096, H=2048, 16 heads, d=128).

Tensor-parallel over heads: 8 NeuronCores x 2 heads each (SPMD).

v2 design (fp8 DoubleRow + unit-interleaved schedule):
  - QKV and w_o projections as split-precision fp8 (hi=e4m3 + lo=e5m2,
    3 cross terms) in DoubleRow perf mode: 2 k-tiles contracted per
    instruction at 0.5 cycles/row -> 0.75x the fp16 PE cost, ~0.2% error.
    Weights pre-scaled by 32 on host (e4m3 range); undone via the exp scale
    (1024x on scores) and a final host-side 1/256.
  - Attention S/PV matmuls stay fp16.
  - Softmax denominator: exp tiles accumulated on DVE into 3 fp16
    accumulators, partition-reduced on gpsimd -- no PE work, no PSUM bank.
  - Attention output normalized + split to (e4m3, e5m2) on the fly.
  - Single interleaved schedule: per stage, attention kt-steps for chunk
    c-1, QKV chain segments for chunk c, and w_o PSUM groups for chunk c-2
    are emitted round-robin, so the PE always has independent matmuls to
    hide exp latency and PSUM-evict latency. hid DMA layout is
    chunk-contiguous; constants load on 3 parallel queues.
"""

import numpy as np
import ml_dtypes

import concourse.tile as tile
from concourse import bacc, bass_isa, mybir
from concourse.bass_utils import run_bass_kernel_spmd

T = 4096
HID = 2048
NHEADS = 16
HD = 128
NCORES = 8
HPC = NHEADS // NCORES        # 2 heads per core
FEAT = HPC * HD               # 256 per-core attention features
QKVF = 3 * FEAT               # 768 per-core qkv features
CH = 512                      # T-chunk width
NCH = T // CH                 # 8 chunks
KT = HID // 128               # 16 hidden k-tiles
NKP = KT // 2                 # 8 DoubleRow k-pairs
SCALE = float(HD) ** -0.5
WSCALE = 32.0                 # host pre-scale on w_qkv and w_o (e4m3 range)
ATSCALE = 8.0                 # scale on normalized attention output
MASK_NEG = -3.0e7             # scores carry a 1024x factor; must still kill exp

F16 = mybir.dt.float16
F32 = mybir.dt.float32
E4 = mybir.dt.float8e4
E5 = mybir.dt.float8e5
NE4 = ml_dtypes.float8_e4m3
NE5 = ml_dtypes.float8_e5m2
DR = mybir.MatmulPerfMode.DoubleRow
MUL = mybir.AluOpType.mult


def _build_bass():
    nc = bacc.Bacc("TRN2", target_bir_lowering=False, debug=False,
                   num_devices=NCORES)

    # hid_*: chunk-contiguous layout [128, NCH, KT, CH] so each chunk loads
    # as one 8KB-per-partition contiguous descriptor.
    hid_hi = nc.dram_tensor("hid_hi", [128, NCH, KT, CH], E4, kind="ExternalInput").ap()
    hid_lo = nc.dram_tensor("hid_lo", [128, NCH, KT, CH], E5, kind="ExternalInput").ap()
    # qkv weights split into A (q0 q1 k0 k1 cols) / B (v cols) so the first
    # chains can start after a smaller DMA on the serial DMA device
    wq_hi_a = nc.dram_tensor("wq_hi_a", [128, KT, 4 * HD], E4, kind="ExternalInput").ap()
    wq_lo_a = nc.dram_tensor("wq_lo_a", [128, KT, 4 * HD], E5, kind="ExternalInput").ap()
    wq_hi_b = nc.dram_tensor("wq_hi_b", [128, KT, FEAT], E4, kind="ExternalInput").ap()
    wq_lo_b = nc.dram_tensor("wq_lo_b", [128, KT, FEAT], E5, kind="ExternalInput").ap()
    wo_hi = nc.dram_tensor("wo_hi", [128, HPC, HID], E4, kind="ExternalInput").ap()
    wo_lo = nc.dram_tensor("wo_lo", [128, HPC, HID], E5, kind="ExternalInput").ap()
    cos2 = nc.dram_tensor("cos2", [128, T], F16, kind="ExternalInput").ap()
    sin2 = nc.dram_tensor("sin2", [128, T], F16, kind="ExternalInput").ap()
    # causal-mask-as-matmul constants: ltri[j,k] = MASK_NEG for j < k,
    # eyepad = [I_128 | 0]; mask block = ltri.T @ eyepad written into PSUM
    # as the opener of each diagonal S accumulation group.
    ltri = nc.dram_tensor("ltri", [128, 128], mybir.dt.bfloat16,
                          kind="ExternalInput").ap()
    eyepad = nc.dram_tensor("eyepad", [128, CH], mybir.dt.bfloat16,
                            kind="ExternalInput").ap()
    out = nc.dram_tensor("out", [T, HID], F16, kind="ExternalOutput").ap()

    with tile.TileContext(nc) as tc:
        _emit(tc, hid_hi, hid_lo, wq_hi_a, wq_lo_a, wq_hi_b, wq_lo_b,
              wo_hi, wo_lo, cos2, sin2, ltri, eyepad, out)
    nc.compile()
    return nc


def _emit(tc, hid_hi, hid_lo, wq_hi_a, wq_lo_a, wq_hi_b, wq_lo_b,
          wo_hi, wo_lo, cos2, sin2, ltri, eyepad, out):
    nc = tc.nc
    from contextlib import ExitStack
    ctx = ExitStack()
    with ctx:
        const = ctx.enter_context(tc.tile_pool(name="const", bufs=1))
        hidp = ctx.enter_context(tc.tile_pool(name="hidp", bufs=2))
        ropep = ctx.enter_context(tc.tile_pool(name="ropep", bufs=4))
        persist = ctx.enter_context(tc.tile_pool(name="persist", bufs=1))
        ptp = ctx.enter_context(tc.tile_pool(name="ptp", bufs=8))
        accp = ctx.enter_context(tc.tile_pool(name="accp", bufs=6))
        nrmp = ctx.enter_context(tc.tile_pool(name="nrmp", bufs=2))
        stgp = ctx.enter_context(tc.tile_pool(name="stgp", bufs=4))
        # PSUM: 8 banks: qkv 2, S 2, po 2, wo 2
        psqkv = ctx.enter_context(tc.tile_pool(name="psqkv", bufs=2, space="PSUM"))
        pss = ctx.enter_context(tc.tile_pool(name="pss", bufs=2, space="PSUM"))
        pso = ctx.enter_context(tc.tile_pool(name="pso", bufs=2, space="PSUM"))
        pswo = ctx.enter_context(tc.tile_pool(name="pswo", bufs=2, space="PSUM"))

        # ---- constants; load order tuned for the serial DMA device ----
        wqh_a = const.tile([128, KT, 4 * HD], E4, tag="wqh_a")
        wql_a = const.tile([128, KT, 4 * HD], E5, tag="wql_a")
        wqh_b = const.tile([128, KT, FEAT], E4, tag="wqh_b")
        wql_b = const.tile([128, KT, FEAT], E5, tag="wql_b")
        woh = const.tile([128, HPC, HID], E4, tag="woh")
        wol = const.tile([128, HPC, HID], E5, tag="wol")
        cos_sb = const.tile([128, T], F16, tag="cos_sb")
        sin_sb = const.tile([128, T], F16, tag="sin_sb")
        ltri_sb = const.tile([128, 128], mybir.dt.bfloat16, tag="ltri_sb")
        eye_sb = const.tile([128, CH], mybir.dt.bfloat16, tag="eye_sb")

        def load_early_consts():
            # order on the (serial) DMA device: wqh_a already issued before
            # hid lo; wql_a next; the V columns (B) arrive later.
            nc.scalar.dma_start(wqh_a[:], wq_hi_a[:])
            nc.sync.dma_start(wql_a[:], wq_lo_a[:])
            nc.scalar.dma_start(wqh_b[:], wq_hi_b[:])
            nc.sync.dma_start(wql_b[:], wq_lo_b[:])

        def load_late_consts():
            nc.gpsimd.dma_start(cos_sb[:], cos2[:])
            nc.gpsimd.dma_start(sin_sb[:], sin2[:])
            nc.scalar.dma_start(ltri_sb[:], ltri[:])
            nc.scalar.dma_start(eye_sb[:], eyepad[:])
            nc.gpsimd.dma_start(woh[:], wo_hi[:])
            nc.gpsimd.dma_start(wol[:], wo_lo[:])

        # ---- persistent activation tiles ----
        QTR = [[persist.tile([128, CH], F16, tag=f"qtr{h}_{c}", name=f"qtr{h}_{c}")
                for c in range(NCH)] for h in range(HPC)]
        KTR = [[persist.tile([128, CH], F16, tag=f"ktr{h}_{c}", name=f"ktr{h}_{c}")
                for c in range(NCH)] for h in range(HPC)]
        VV = persist.tile([128, HPC * T], F16, tag="vv", name="vv")
        ATH = [persist.tile([128, HPC, CH], E4, tag=f"ath{c}", name=f"ath{c}")
               for c in range(NCH)]
        ATL = [persist.tile([128, HPC, CH], E5, tag=f"atl{c}", name=f"atl{c}")
               for c in range(NCH)]

        hid_tiles = {}

        def load_hid(c):
            hh = hidp.tile([128, KT, CH], E4, tag="hh", name=f"hh{c}")
            hl = hidp.tile([128, KT, CH], E5, tag="hl", name=f"hl{c}")
            nc.gpsimd.dma_start(hh[:], hid_hi[:, c, :, :])
            nc.gpsimd.dma_start(hl[:], hid_lo[:, c, :, :])
            hid_tiles[c] = (hh, hl)

        deferred_rope = []

        def rope_close(ps, ft, c):
            raw = ropep.tile([128, CH], F16, tag="raw", name=f"raw{c}_{ft}")
            nc.scalar.copy(raw[:], ps[:])
            rot = ropep.tile([128, CH], F16, tag="rot", name=f"rot{c}_{ft}")
            nc.sync.dma_start(rot[0:64, :], raw[64:128, :])
            nc.sync.dma_start(rot[64:128, :], raw[0:64, :])
            deferred_rope.append((raw, rot, ft, c))

        def flush_rope():
            for raw, rot, ft, c in deferred_rope:
                h = ft % 2
                ta = ropep.tile([128, CH], F16, tag="ta", name=f"ta{c}_{ft}")
                tb = ropep.tile([128, CH], F16, tag="tb", name=f"tb{c}_{ft}")
                csl = slice(c * CH, (c + 1) * CH)
                nc.vector.tensor_mul(ta[:], raw[:], cos_sb[:, csl])
                nc.vector.tensor_mul(tb[:], rot[:], sin_sb[:, csl])
                dst = QTR[h][c] if ft < 2 else KTR[h][c]
                nc.vector.tensor_add(dst[:], ta[:], tb[:])
            deferred_rope.clear()

        # ------- QKV chain units: 3 fp8-split passes over one chain -------
        PASSES = ((0, 0), (0, 1), (1, 0))   # (w lo?, hid lo?)

        def qkv_units(c):
            """Yield (closure, is_last_of_chain) for chunk c's 8 chains in
            pair-interleaved order (2 chains in flight on 2 PSUM banks)."""
            hh, hl = hid_tiles[c]
            state = {}

            def qk_seg(ft, p):
                def run():
                    if (ft, 'ps') not in state:
                        state[(ft, 'ps')] = psqkv.tile(
                            [128, CH], F32, tag="qkv", name=f"psq{c}_{ft}")
                    ps = state[(ft, 'ps')]
                    wt = wql_a if PASSES[p][0] else wqh_a
                    xt = hl if PASSES[p][1] else hh
                    fsl = slice(ft * 128, (ft + 1) * 128)
                    for kp in range(NKP):
                        ksl = slice(2 * kp, 2 * kp + 2)
                        nc.tensor.matmul(ps[:], wt[:, ksl, fsl], xt[:, ksl, :],
                                         start=(p == 0 and kp == 0),
                                         stop=(p == 2 and kp == NKP - 1),
                                         perf_mode=DR)
                    if p == 2:
                        rope_close(ps, ft, c)
                return run

            def v_seg(j, p):
                def run():
                    if (4 + j, 'ps') not in state:
                        state[(4 + j, 'ps')] = psqkv.tile(
                            [128, CH], F32, tag="qkv", name=f"psv{c}_{j}")
                    ps = state[(4 + j, 'ps')]
                    wt = wql_b if PASSES[p][0] else wqh_b
                    xt = hl if PASSES[p][1] else hh
                    jsl = slice(j * 128, (j + 1) * 128)
                    for kp in range(NKP):
                        ksl = slice(2 * kp, 2 * kp + 2)
                        nc.tensor.matmul(ps[:, :FEAT],
                                         xt[:, ksl, jsl], wt[:, ksl, :],
                                         start=(p == 0 and kp == 0),
                                         stop=(p == 2 and kp == NKP - 1),
                                         perf_mode=DR)
                    if p == 2:
                        kt_ = 4 * c + j
                        nc.scalar.copy(VV[:, kt_ * FEAT:(kt_ + 1) * FEAT],
                                       ps[:, :FEAT])
                return run

            # chain pairs: q/k first (only need the A weight columns)
            pairs = [(qk_seg, 0, qk_seg, 2), (qk_seg, 1, qk_seg, 3),
                     (v_seg, 0, v_seg, 1), (v_seg, 2, v_seg, 3)]
            for fa, ia, fb, ib in pairs:
                for p in range(3):
                    yield fa(ia, p)
                    yield fb(ib, p)

        # ------------- attention units for one (chunk, head) -------------
        def attn_units(c, h):
            nkt = 4 * (c + 1)
            st = {}

            def setup():
                st['po'] = pso.tile([128, CH], F32, tag="o", name=f"po{h}_{c}")
                st['acc'] = [accp.tile([128, CH], F16, tag="acc",
                                       name=f"acc{h}_{c}_{i}") for i in range(3)]
                st['first'] = [c > 0, c > 0, c > 0]
                if c == 0:
                    for a in st['acc']:
                        nc.any.memset(a[:], 0.0)
                st['pending'] = []

            def s_exp(kt):
                r = kt - 4 * c
                qo = 128 * r if r > 0 else 0
                ps = pss.tile([128, CH], F32, tag="s", name=f"ps{h}_{c}_{kt}")
                if r >= 0:
                    # open the group with the causal mask: ltri.T @ [I|0]
                    nc.tensor.matmul(ps[:, qo:], ltri_sb[:],
                                     eye_sb[:, :CH - qo],
                                     start=True, stop=False)
                nc.tensor.matmul(
                    ps[:, qo:],
                    KTR[h][kt // 4][:, (kt % 4) * 128:(kt % 4 + 1) * 128],
                    QTR[h][c][:, qo:],
                    start=(r < 0), stop=True)
                pt = ptp.tile([128, CH], F16, tag="pt", name=f"pt{h}_{c}_{kt}")
                nc.scalar.activation(pt[:, qo:], ps[:, qo:],
                                     mybir.ActivationFunctionType.Exp,
                                     scale=SCALE / (WSCALE * WSCALE))
                return kt, qo, pt

            def pv_acc(pend):
                kt, qo, pt = pend
                nc.tensor.matmul(
                    st['po'][:, qo:],
                    VV[:, kt * FEAT + h * 128:kt * FEAT + (h + 1) * 128],
                    pt[:, qo:],
                    start=(kt == 0), stop=(kt == nkt - 1))
                i = kt % 3
                if st['first'][i]:
                    nc.vector.tensor_copy(st['acc'][i][:], pt[:])
                    st['first'][i] = False
                else:
                    nc.vector.tensor_add(st['acc'][i][:, qo:],
                                         st['acc'][i][:, qo:], pt[:, qo:])

            def unit(kt):
                def run():
                    if kt == 0:
                        setup()
                    st['pending'].append(s_exp(kt))
                    if len(st['pending']) > 2:
                        pv_acc(st['pending'].pop(0))
                return run

            def finish():
                for pend in st['pending']:
                    pv_acc(pend)
                st['pending'] = []
                acc = st['acc']
                nc.vector.tensor_add(acc[0][:], acc[0][:], acc[1][:])
                nc.vector.tensor_add(acc[0][:], acc[0][:], acc[2][:])
                pdall = nrmp.tile([128, CH], F32, tag="pdall", name=f"pd{h}_{c}")
                nc.gpsimd.partition_all_reduce(
                    pdall[:], acc[0][:], channels=128,
                    reduce_op=bass_isa.ReduceOp.add)
                binv = nrmp.tile([128, CH], F32, tag="binv", name=f"bi{h}_{c}")
                nc.vector.reciprocal(binv[:], pdall[:])
                at16 = nrmp.tile([128, CH], F16, tag="at16", name=f"a16{h}_{c}")
                nc.vector.scalar_tensor_tensor(
                    at16[:], st['po'][:], ATSCALE / WSCALE, binv[:],
                    op0=MUL, op1=MUL)
                nc.vector.tensor_copy(ATH[c][:, h, :], at16[:])
                nc.vector.tensor_sub(ATL[c][:, h, :], at16[:], ATH[c][:, h, :])

            return [unit(kt) for kt in range(nkt)], finish

        # --------- w_o units (fp8 split DoubleRow, head-paired) ----------
        def wo_units(c, split_last_dma=False):
            stt = {}

            def unit(j, n):
                def run():
                    tt = 4 * c + j
                    if n == 0:
                        stt[j] = stgp.tile([128, HID], F16, tag="stg",
                                           name=f"stg{tt}")
                    stg = stt[j]
                    jsl = slice(j * 128, (j + 1) * 128)
                    nsl = slice(n * CH, (n + 1) * CH)
                    pw = pswo.tile([128, CH], F32, tag="w", name=f"pw{tt}_{n}")
                    for i, (a, w) in enumerate(
                            ((ATH[c], woh), (ATH[c], wol), (ATL[c], woh))):
                        nc.tensor.matmul(pw[:], a[:, :, jsl], w[:, :, nsl],
                                         start=(i == 0), stop=(i == 2),
                                         perf_mode=DR)
                    if (n + tt) % 4 < 2:
                        nc.vector.tensor_copy(stg[:, nsl], pw[:])
                    else:
                        nc.scalar.copy(stg[:, nsl], pw[:])
                    if split_last_dma:
                        # stream each quarter out as soon as it is staged
                        eng = (nc.sync, nc.scalar, nc.gpsimd, nc.sync)[n]
                        eng.dma_start(out[tt * 128:(tt + 1) * 128, nsl],
                                      stg[:, nsl])
                    elif n == 3:
                        eng = nc.sync if tt % 2 == 0 else nc.scalar
                        eng.dma_start(out[tt * 128:(tt + 1) * 128, :], stg[:])
                return run

            return [unit(j, n) for j in range(4) for n in range(4)]

        def weave(primary, fillers):
            """Emit primary units with fillers spread evenly between them."""
            np_, nf = len(primary), len(fillers)
            fi = 0
            for i, u in enumerate(primary):
                u()
                want = (i + 1) * nf // np_
                while fi < want:
                    fillers[fi]()
                    fi += 1
            while fi < nf:
                fillers[fi]()
                fi += 1

        def mix(a, b):
            """Merge two unit lists, spreading b evenly through a."""
            if not b:
                return list(a)
            if not a:
                return list(b)
            res = []
            bi = 0
            for i, u in enumerate(a):
                res.append(u)
                want = (i + 1) * len(b) // len(a)
                while bi < want:
                    res.append(b[bi])
                    bi += 1
            res.extend(b[bi:])
            return res

        # ================= main schedule =================
        load_hid(0)
        load_early_consts()
        load_hid(1)
        load_late_consts()
        for u in qkv_units(0):
            u()
        flush_rope()

        for k in range(1, NCH + 1):
            c_attn = k - 1
            c_wo = k - 2
            if k < NCH:
                if k + 1 < NCH:
                    load_hid(k + 1)
                h0_units, h0_fin = attn_units(c_attn, 0)
                h1_units, h1_fin = attn_units(c_attn, 1)
                qun = list(qkv_units(k))
                woun = wo_units(c_wo) if c_wo >= 0 else []
                # dense blocks (fine weaving micro-gaps the PE and resets
                # its p-state ramp): attention, qkv, attention, qkv, wo.
                for u in h0_units:
                    u()
                h0_fin()
                for u in qun[:12]:      # (q0,k0), (v0,v1) chain pairs
                    u()
                for u in h1_units:
                    u()
                for u in qun[12:]:      # (v2,v3), (q1,k1) chain pairs
                    u()
                for u in woun:
                    u()
                flush_rope()
            else:
                h0_units, h0_fin = attn_units(c_attn, 0)
                h1_units, h1_fin = attn_units(c_attn, 1)
                woun = wo_units(c_wo)
                weave(h0_units, woun[:8])
                h0_fin()
                weave(h1_units, woun[8:])
                h1_fin()
        for u in wo_units(NCH - 1, split_last_dma=True):
            u()


_NC_CACHE = None


def _get_nc():
    global _NC_CACHE
    if _NC_CACHE is None:
        _NC_CACHE = _build_bass()
    return _NC_CACHE


def _split8(x):
    hi = np.ascontiguousarray(x).astype(NE4)
    lo = (x - hi.astype(np.float32)).astype(NE5)
    return hi, np.ascontiguousarray(lo)


def prepare_inputs(hidden_states, positions, w_qkv, w_o):
    """Host-side sharding/preprocessing -> list of per-core input maps."""
    hidden_states = np.asarray(hidden_states, dtype=np.float32)
    positions = np.asarray(positions)
    w_qkv = np.asarray(w_qkv, dtype=np.float32)
    w_o = np.asarray(w_o, dtype=np.float32)

    # hidden^T -> [128, NCH, KT, CH] chunk-contiguous fp8 hi/lo
    hidT = (hidden_states.T.reshape(KT, 128, NCH, CH)
            .transpose(1, 2, 0, 3))
    hid_hi, hid_lo = _split8(hidT)

    pos = positions.astype(np.float32)
    half = HD // 2
    inv_freq = 1.0 / (10000.0 ** (np.arange(half, dtype=np.float32) / half))
    freqs = np.outer(pos, inv_freq)          # [T, 64]
    cos = np.cos(freqs).T                    # [64, T]
    sin = np.sin(freqs).T
    cos2 = np.concatenate([cos, cos], axis=0).astype(np.float16)
    sin2 = np.concatenate([-sin, sin], axis=0).astype(np.float16)

    j_idx = np.arange(128)[:, None]
    k_idx = np.arange(128)[None, :]
    ltri_np = np.where(j_idx < k_idx, MASK_NEG, 0.0).astype(ml_dtypes.bfloat16)
    eye_np = np.zeros((128, CH), dtype=ml_dtypes.bfloat16)
    eye_np[:, :128] = np.eye(128)

    in_maps = []
    for core in range(NCORES):
        heads = [HPC * core + i for i in range(HPC)]
        wq = [w_qkv[:, h * HD:(h + 1) * HD] for h in heads]
        wk = [w_qkv[:, FEAT * NCORES + h * HD:FEAT * NCORES + (h + 1) * HD]
              for h in heads]
        wv = [w_qkv[:, 2 * FEAT * NCORES + h * HD:2 * FEAT * NCORES + (h + 1) * HD]
              for h in heads]
        wqkv_core = np.concatenate(wq + wk + wv, axis=1) * WSCALE
        wqkv_core = wqkv_core.reshape(KT, 128, QKVF).transpose(1, 0, 2)
        wq_hi, wq_lo = _split8(wqkv_core)
        wq_hi_a = np.ascontiguousarray(wq_hi[:, :, :4 * HD])
        wq_hi_b = np.ascontiguousarray(wq_hi[:, :, 4 * HD:])
        wq_lo_a = np.ascontiguousarray(wq_lo[:, :, :4 * HD])
        wq_lo_b = np.ascontiguousarray(wq_lo[:, :, 4 * HD:])
        wo_core = np.stack(
            [w_o[h * HD:(h + 1) * HD, :] for h in heads], axis=0) * WSCALE
        wo_core = wo_core.transpose(1, 0, 2)   # [128, HPC, HID]
        wo_hi, wo_lo = _split8(wo_core)
        in_maps.append({
            "hid_hi": hid_hi,
            "hid_lo": hid_lo,
            "wq_hi_a": wq_hi_a,
            "wq_lo_a": wq_lo_a,
            "wq_hi_b": wq_hi_b,
            "wq_lo_b": wq_lo_b,
            "wo_hi": wo_hi,
            "wo_lo": wo_lo,
            "cos2": cos2,
            "sin2": sin2,
            "ltri": ltri_np,
            "eyepad": eye_np,
        })
    return in_maps


def kernel(hidden_states, positions, w_qkv, w_o):
    in_maps = prepare_inputs(hidden_states, positions, w_qkv, w_o)
    nc = _get_nc()
    try:
        res = run_bass_kernel_spmd(nc, in_maps, core_ids=list(range(NCORES)))
    except Exception:
        # transient device wedge from a prior crashed process: retry once
        res = run_bass_kernel_spmd(nc, in_maps, core_ids=list(range(NCORES)))
    acc = res.results[0]["out"].astype(np.float32)
    for i in range(1, NCORES):
        acc += res.results[i]["out"].astype(np.float32)
    return acc * (1.0 / (ATSCALE * WSCALE))


# revision 32
# speedup vs baseline: 1.1366x; 1.0010x over previous
"""Trainium2 Bass kernel for DeepseekAttention (T=4096, H=2048, 16 heads, d=128).

Tensor-parallel over heads: 8 NeuronCores x 2 heads each (SPMD).

v2 design (fp8 DoubleRow + unit-interleaved schedule):
  - QKV and w_o projections as split-precision fp8 (hi=e4m3 + lo=e5m2,
    3 cross terms) in DoubleRow perf mode: 2 k-tiles contracted per
    instruction at 0.5 cycles/row -> 0.75x the fp16 PE cost, ~0.2% error.
    Weights pre-scaled by 32 on host (e4m3 range); undone via the exp scale
    (1024x on scores) and a final host-side 1/256.
  - Attention S/PV matmuls stay fp16.
  - Softmax denominator: exp tiles accumulated on DVE into 3 fp16
    accumulators, partition-reduced on gpsimd -- no PE work, no PSUM bank.
  - Attention output normalized + split to (e4m3, e5m2) on the fly.
  - Single interleaved schedule: per stage, attention kt-steps for chunk
    c-1, QKV chain segments for chunk c, and w_o PSUM groups for chunk c-2
    are emitted round-robin, so the PE always has independent matmuls to
    hide exp latency and PSUM-evict latency. hid DMA layout is
    chunk-contiguous; constants load on 3 parallel queues.
"""

import numpy as np
import ml_dtypes

import concourse.tile as tile
from concourse import bacc, bass_isa, mybir
from concourse.bass_utils import run_bass_kernel_spmd

T = 4096
HID = 2048
NHEADS = 16
HD = 128
NCORES = 8
HPC = NHEADS // NCORES        # 2 heads per core
FEAT = HPC * HD               # 256 per-core attention features
QKVF = 3 * FEAT               # 768 per-core qkv features
CH = 512                      # T-chunk width
NCH = T // CH                 # 8 chunks
KT = HID // 128               # 16 hidden k-tiles
NKP = KT // 2                 # 8 DoubleRow k-pairs
SCALE = float(HD) ** -0.5
WSCALE = 32.0                 # host pre-scale on w_qkv and w_o (e4m3 range)
ATSCALE = 8.0                 # scale on normalized attention output
MASK_NEG = -3.0e7             # scores carry a 1024x factor; must still kill exp

F16 = mybir.dt.float16
F32 = mybir.dt.float32
E4 = mybir.dt.float8e4
E5 = mybir.dt.float8e5
NE4 = ml_dtypes.float8_e4m3
NE5 = ml_dtypes.float8_e5m2
DR = mybir.MatmulPerfMode.DoubleRow
MUL = mybir.AluOpType.mult


def _build_bass():
    nc = bacc.Bacc("TRN2", target_bir_lowering=False, debug=False,
                   num_devices=NCORES)

    # hid_*: chunk-contiguous layout [128, NCH, KT, CH] so each chunk loads
    # as one 8KB-per-partition contiguous descriptor.
    hid_hi = nc.dram_tensor("hid_hi", [128, NCH, KT, CH], E4, kind="ExternalInput").ap()
    hid_lo = nc.dram_tensor("hid_lo", [128, NCH, KT, CH], E5, kind="ExternalInput").ap()
    # qkv weights split into A (q0 q1 k0 k1 cols) / B (v cols) so the first
    # chains can start after a smaller DMA on the serial DMA device
    wq_hi_a = nc.dram_tensor("wq_hi_a", [128, KT, 4 * HD], E4, kind="ExternalInput").ap()
    wq_lo_a = nc.dram_tensor("wq_lo_a", [128, KT, 4 * HD], E5, kind="ExternalInput").ap()
    wq_hi_b = nc.dram_tensor("wq_hi_b", [128, KT, FEAT], E4, kind="ExternalInput").ap()
    wq_lo_b = nc.dram_tensor("wq_lo_b", [128, KT, FEAT], E5, kind="ExternalInput").ap()
    wo_hi = nc.dram_tensor("wo_hi", [128, HPC, HID], E4, kind="ExternalInput").ap()
    wo_lo = nc.dram_tensor("wo_lo", [128, HPC, HID], E5, kind="ExternalInput").ap()
    cos2 = nc.dram_tensor("cos2", [128, T], F16, kind="ExternalInput").ap()
    sin2 = nc.dram_tensor("sin2", [128, T], F16, kind="ExternalInput").ap()
    # causal-mask-as-matmul constants: ltri[j,k] = MASK_NEG for j < k,
    # eyepad = [I_128 | 0]; mask block = ltri.T @ eyepad written into PSUM
    # as the opener of each diagonal S accumulation group.
    ltri = nc.dram_tensor("ltri", [128, 128], mybir.dt.bfloat16,
                          kind="ExternalInput").ap()
    eyepad = nc.dram_tensor("eyepad", [128, CH], mybir.dt.bfloat16,
                            kind="ExternalInput").ap()
    out = nc.dram_tensor("out", [T, HID], F16, kind="ExternalOutput").ap()

    with tile.TileContext(nc) as tc:
        _emit(tc, hid_hi, hid_lo, wq_hi_a, wq_lo_a, wq_hi_b, wq_lo_b,
              wo_hi, wo_lo, cos2, sin2, ltri, eyepad, out)
    nc.compile()
    return nc


def _emit(tc, hid_hi, hid_lo, wq_hi_a, wq_lo_a, wq_hi_b, wq_lo_b,
          wo_hi, wo_lo, cos2, sin2, ltri, eyepad, out):
    nc = tc.nc
    from contextlib import ExitStack
    ctx = ExitStack()
    with ctx:
        const = ctx.enter_context(tc.tile_pool(name="const", bufs=1))
        hidp = ctx.enter_context(tc.tile_pool(name="hidp", bufs=2))
        ropep = ctx.enter_context(tc.tile_pool(name="ropep", bufs=4))
        persist = ctx.enter_context(tc.tile_pool(name="persist", bufs=1))
        ptp = ctx.enter_context(tc.tile_pool(name="ptp", bufs=8))
        accp = ctx.enter_context(tc.tile_pool(name="accp", bufs=6))
        nrmp = ctx.enter_context(tc.tile_pool(name="nrmp", bufs=2))
        stgp = ctx.enter_context(tc.tile_pool(name="stgp", bufs=4))
        # PSUM: 8 banks: qkv 2, S 2, po 2, wo 2
        psqkv = ctx.enter_context(tc.tile_pool(name="psqkv", bufs=2, space="PSUM"))
        pss = ctx.enter_context(tc.tile_pool(name="pss", bufs=2, space="PSUM"))
        pso = ctx.enter_context(tc.tile_pool(name="pso", bufs=2, space="PSUM"))
        pswo = ctx.enter_context(tc.tile_pool(name="pswo", bufs=2, space="PSUM"))

        # ---- constants; load order tuned for the serial DMA device ----
        wqh_a = const.tile([128, KT, 4 * HD], E4, tag="wqh_a")
        wql_a = const.tile([128, KT, 4 * HD], E5, tag="wql_a")
        wqh_b = const.tile([128, KT, FEAT], E4, tag="wqh_b")
        wql_b = const.tile([128, KT, FEAT], E5, tag="wql_b")
        woh = const.tile([128, HPC, HID], E4, tag="woh")
        wol = const.tile([128, HPC, HID], E5, tag="wol")
        cos_sb = const.tile([128, T], F16, tag="cos_sb")
        sin_sb = const.tile([128, T], F16, tag="sin_sb")
        ltri_sb = const.tile([128, 128], mybir.dt.bfloat16, tag="ltri_sb")
        eye_sb = const.tile([128, CH], mybir.dt.bfloat16, tag="eye_sb")

        def load_early_consts():
            # order on the (serial) DMA device: wqh_a already issued before
            # hid lo; wql_a next; the V columns (B) arrive later.
            nc.scalar.dma_start(wqh_a[:], wq_hi_a[:])
            nc.sync.dma_start(wql_a[:], wq_lo_a[:])
            nc.scalar.dma_start(wqh_b[:], wq_hi_b[:])
            nc.sync.dma_start(wql_b[:], wq_lo_b[:])

        def load_late_consts():
            nc.gpsimd.dma_start(cos_sb[:], cos2[:])
            nc.gpsimd.dma_start(sin_sb[:], sin2[:])
            nc.scalar.dma_start(ltri_sb[:], ltri[:])
            nc.scalar.dma_start(eye_sb[:], eyepad[:])
            nc.gpsimd.dma_start(woh[:], wo_hi[:])
            nc.gpsimd.dma_start(wol[:], wo_lo[:])

        # ---- persistent activation tiles ----
        QTR = [[persist.tile([128, CH], F16, tag=f"qtr{h}_{c}", name=f"qtr{h}_{c}")
                for c in range(NCH)] for h in range(HPC)]
        KTR = [[persist.tile([128, CH], F16, tag=f"ktr{h}_{c}", name=f"ktr{h}_{c}")
                for c in range(NCH)] for h in range(HPC)]
        VV = persist.tile([128, HPC * T], F16, tag="vv", name="vv")
        ATH = [persist.tile([128, HPC, CH], E4, tag=f"ath{c}", name=f"ath{c}")
               for c in range(NCH)]
        ATL = [persist.tile([128, HPC, CH], E5, tag=f"atl{c}", name=f"atl{c}")
               for c in range(NCH)]

        hid_tiles = {}

        def load_hid(c):
            hh = hidp.tile([128, KT, CH], E4, tag="hh", name=f"hh{c}")
            hl = hidp.tile([128, KT, CH], E5, tag="hl", name=f"hl{c}")
            nc.gpsimd.dma_start(hh[:], hid_hi[:, c, :, :])
            nc.gpsimd.dma_start(hl[:], hid_lo[:, c, :, :])
            hid_tiles[c] = (hh, hl)

        deferred_rope = []

        def rope_close(ps, ft, c):
            raw = ropep.tile([128, CH], F16, tag="raw", name=f"raw{c}_{ft}")
            nc.scalar.copy(raw[:], ps[:])
            rot = ropep.tile([128, CH], F16, tag="rot", name=f"rot{c}_{ft}")
            nc.sync.dma_start(rot[0:64, :], raw[64:128, :])
            nc.sync.dma_start(rot[64:128, :], raw[0:64, :])
            deferred_rope.append((raw, rot, ft, c))

        def flush_rope():
            for raw, rot, ft, c in deferred_rope:
                h = ft % 2
                ta = ropep.tile([128, CH], F16, tag="ta", name=f"ta{c}_{ft}")
                tb = ropep.tile([128, CH], F16, tag="tb", name=f"tb{c}_{ft}")
                csl = slice(c * CH, (c + 1) * CH)
                nc.vector.tensor_mul(ta[:], raw[:], cos_sb[:, csl])
                nc.vector.tensor_mul(tb[:], rot[:], sin_sb[:, csl])
                dst = QTR[h][c] if ft < 2 else KTR[h][c]
                nc.vector.tensor_add(dst[:], ta[:], tb[:])
            deferred_rope.clear()

        # ------- QKV chain units: 3 fp8-split passes over one chain -------
        PASSES = ((0, 0), (0, 1), (1, 0))   # (w lo?, hid lo?)

        def qkv_units(c):
            """Yield (closure, is_last_of_chain) for chunk c's 8 chains in
            pair-interleaved order (2 chains in flight on 2 PSUM banks)."""
            hh, hl = hid_tiles[c]
            state = {}

            def qk_seg(ft, p):
                def run():
                    if (ft, 'ps') not in state:
                        state[(ft, 'ps')] = psqkv.tile(
                            [128, CH], F32, tag="qkv", name=f"psq{c}_{ft}")
                    ps = state[(ft, 'ps')]
                    wt = wql_a if PASSES[p][0] else wqh_a
                    xt = hl if PASSES[p][1] else hh
                    fsl = slice(ft * 128, (ft + 1) * 128)
                    for kp in range(NKP):
                        ksl = slice(2 * kp, 2 * kp + 2)
                        nc.tensor.matmul(ps[:], wt[:, ksl, fsl], xt[:, ksl, :],
                                         start=(p == 0 and kp == 0),
                                         stop=(p == 2 and kp == NKP - 1),
                                         perf_mode=DR)
                    if p == 2:
                        rope_close(ps, ft, c)
                return run

            def v_seg(j, p):
                def run():
                    if (4 + j, 'ps') not in state:
                        state[(4 + j, 'ps')] = psqkv.tile(
                            [128, CH], F32, tag="qkv", name=f"psv{c}_{j}")
                    ps = state[(4 + j, 'ps')]
                    wt = wql_b if PASSES[p][0] else wqh_b
                    xt = hl if PASSES[p][1] else hh
                    jsl = slice(j * 128, (j + 1) * 128)
                    for kp in range(NKP):
                        ksl = slice(2 * kp, 2 * kp + 2)
                        nc.tensor.matmul(ps[:, :FEAT],
                                         xt[:, ksl, jsl], wt[:, ksl, :],
                                         start=(p == 0 and kp == 0),
                                         stop=(p == 2 and kp == NKP - 1),
                                         perf_mode=DR)
                    if p == 2:
                        kt_ = 4 * c + j
                        nc.scalar.copy(VV[:, kt_ * FEAT:(kt_ + 1) * FEAT],
                                       ps[:, :FEAT])
                return run

            # chain pairs: q/k first (only need the A weight columns)
            pairs = [(qk_seg, 0, qk_seg, 2), (qk_seg, 1, qk_seg, 3),
                     (v_seg, 0, v_seg, 1), (v_seg, 2, v_seg, 3)]
            for fa, ia, fb, ib in pairs:
                for p in range(3):
                    yield fa(ia, p)
                    yield fb(ib, p)

        # ------------- attention units for one (chunk, head) -------------
        def attn_units(c, h):
            nkt = 4 * (c + 1)
            st = {}

            def setup():
                st['po'] = pso.tile([128, CH], F32, tag="o", name=f"po{h}_{c}")
                st['acc'] = [accp.tile([128, CH], F16, tag="acc",
                                       name=f"acc{h}_{c}_{i}") for i in range(3)]
                st['first'] = [c > 0, c > 0, c > 0]
                if c == 0:
                    for a in st['acc']:
                        nc.any.memset(a[:], 0.0)
                st['pending'] = []

            def s_exp(kt):
                r = kt - 4 * c
                qo = 128 * r if r > 0 else 0
                ps = pss.tile([128, CH], F32, tag="s", name=f"ps{h}_{c}_{kt}")
                if r >= 0:
                    # open the group with the causal mask: ltri.T @ [I|0]
                    nc.tensor.matmul(ps[:, qo:], ltri_sb[:],
                                     eye_sb[:, :CH - qo],
                                     start=True, stop=False)
                nc.tensor.matmul(
                    ps[:, qo:],
                    KTR[h][kt // 4][:, (kt % 4) * 128:(kt % 4 + 1) * 128],
                    QTR[h][c][:, qo:],
                    start=(r < 0), stop=True)
                pt = ptp.tile([128, CH], F16, tag="pt", name=f"pt{h}_{c}_{kt}")
                nc.scalar.activation(pt[:, qo:], ps[:, qo:],
                                     mybir.ActivationFunctionType.Exp,
                                     scale=SCALE / (WSCALE * WSCALE))
                return kt, qo, pt

            def pv(pend):
                kt, qo, pt = pend
                nc.tensor.matmul(
                    st['po'][:, qo:],
                    VV[:, kt * FEAT + h * 128:kt * FEAT + (h + 1) * 128],
                    pt[:, qo:],
                    start=(kt == 0), stop=(kt == nkt - 1))

            def acc_add(pend):
                kt, qo, pt = pend
                i = kt % 3
                if st['first'][i]:
                    nc.vector.tensor_copy(st['acc'][i][:], pt[:])
                    st['first'][i] = False
                else:
                    nc.vector.tensor_add(st['acc'][i][:, qo:],
                                         st['acc'][i][:, qo:], pt[:, qo:])

            def pv_acc(pend):
                pv(pend)
                acc_add(pend)

            def unit(kt):
                def run():
                    if kt == 0:
                        setup()
                    st['pending'].append(s_exp(kt))
                    if len(st['pending']) > 2:
                        pv_acc(st['pending'].pop(0))
                return run

            def finish():
                # denominator chain first (needs only exps), PVs in parallel
                for pend in st['pending']:
                    acc_add(pend)
                acc = st['acc']
                nc.vector.tensor_add(acc[0][:], acc[0][:], acc[1][:])
                nc.vector.tensor_add(acc[0][:], acc[0][:], acc[2][:])
                pdall = nrmp.tile([128, CH], F32, tag="pdall", name=f"pd{h}_{c}")
                nc.gpsimd.partition_all_reduce(
                    pdall[:], acc[0][:], channels=128,
                    reduce_op=bass_isa.ReduceOp.add)
                binv = nrmp.tile([128, CH], F32, tag="binv", name=f"bi{h}_{c}")
                nc.vector.reciprocal(binv[:], pdall[:])
                for pend in st['pending']:
                    pv(pend)
                st['pending'] = []
                at16 = nrmp.tile([128, CH], F16, tag="at16", name=f"a16{h}_{c}")
                nc.vector.scalar_tensor_tensor(
                    at16[:], st['po'][:], ATSCALE / WSCALE, binv[:],
                    op0=MUL, op1=MUL)
                nc.vector.tensor_copy(ATH[c][:, h, :], at16[:])
                nc.vector.tensor_sub(ATL[c][:, h, :], at16[:], ATH[c][:, h, :])

            return [unit(kt) for kt in range(nkt)], finish

        # --------- w_o units (fp8 split DoubleRow, head-paired) ----------
        def wo_units(c, split_last_dma=False, extra_psum=False):
            stt = {}

            def unit(j, n):
                def run():
                    tt = 4 * c + j
                    if n == 0:
                        stt[j] = stgp.tile([128, HID], F16, tag="stg",
                                           name=f"stg{tt}")
                    stg = stt[j]
                    jsl = slice(j * 128, (j + 1) * 128)
                    nsl = slice(n * CH, (n + 1) * CH)
                    # in the tail the qkv PSUM pool is idle: alternate with
                    # it for an effective ring of 4 accumulators
                    pool = (psqkv if extra_psum and (4 * j + n) % 2 else pswo)
                    tag = "qkv" if pool is psqkv else "w"
                    pw = pool.tile([128, CH], F32, tag=tag, name=f"pw{tt}_{n}")
                    for i, (a, w) in enumerate(
                            ((ATH[c], woh), (ATH[c], wol), (ATL[c], woh))):
                        nc.tensor.matmul(pw[:], a[:, :, jsl], w[:, :, nsl],
                                         start=(i == 0), stop=(i == 2),
                                         perf_mode=DR)
                    if (n + tt) % 4 < 2:
                        nc.vector.tensor_copy(stg[:, nsl], pw[:])
                    else:
                        nc.scalar.copy(stg[:, nsl], pw[:])
                    if split_last_dma:
                        # stream each quarter out as soon as it is staged
                        eng = (nc.sync, nc.scalar, nc.gpsimd, nc.sync)[n]
                        eng.dma_start(out[tt * 128:(tt + 1) * 128, nsl],
                                      stg[:, nsl])
                    elif n == 3:
                        eng = nc.sync if tt % 2 == 0 else nc.scalar
                        eng.dma_start(out[tt * 128:(tt + 1) * 128, :], stg[:])
                return run

            return [unit(j, n) for j in range(4) for n in range(4)]

        def weave(primary, fillers):
            """Emit primary units with fillers spread evenly between them."""
            np_, nf = len(primary), len(fillers)
            fi = 0
            for i, u in enumerate(primary):
                u()
                want = (i + 1) * nf // np_
                while fi < want:
                    fillers[fi]()
                    fi += 1
            while fi < nf:
                fillers[fi]()
                fi += 1

        def mix(a, b):
            """Merge two unit lists, spreading b evenly through a."""
            if not b:
                return list(a)
            if not a:
                return list(b)
            res = []
            bi = 0
            for i, u in enumerate(a):
                res.append(u)
                want = (i + 1) * len(b) // len(a)
                while bi < want:
                    res.append(b[bi])
                    bi += 1
            res.extend(b[bi:])
            return res

        # ================= main schedule =================
        load_hid(0)
        load_early_consts()
        load_hid(1)
        load_late_consts()
        for u in qkv_units(0):
            u()
        flush_rope()

        for k in range(1, NCH + 1):
            c_attn = k - 1
            c_wo = k - 2
            if k < NCH:
                if k + 1 < NCH:
                    load_hid(k + 1)
                h0_units, h0_fin = attn_units(c_attn, 0)
                h1_units, h1_fin = attn_units(c_attn, 1)
                qun = list(qkv_units(k))
                woun = wo_units(c_wo) if c_wo >= 0 else []
                # dense blocks (fine weaving micro-gaps the PE and resets
                # its p-state ramp): attention, qkv, attention, qkv, wo.
                for u in h0_units:
                    u()
                h0_fin()
                for u in qun[:12]:      # (q0,k0), (v0,v1) chain pairs
                    u()
                for u in h1_units:
                    u()
                for u in qun[12:]:      # (v2,v3), (q1,k1) chain pairs
                    u()
                for u in woun:
                    u()
                flush_rope()
            else:
                h0_units, h0_fin = attn_units(c_attn, 0)
                h1_units, h1_fin = attn_units(c_attn, 1)
                woun = wo_units(c_wo, extra_psum=True)
                weave(h0_units, woun[:8])
                h0_fin()
                weave(h1_units, woun[8:])
                h1_fin()
        for u in wo_units(NCH - 1, split_last_dma=True, extra_psum=True):
            u()


_NC_CACHE = None


def _get_nc():
    global _NC_CACHE
    if _NC_CACHE is None:
        _NC_CACHE = _build_bass()
    return _NC_CACHE


def _split8(x):
    hi = np.ascontiguousarray(x).astype(NE4)
    lo = (x - hi.astype(np.float32)).astype(NE5)
    return hi, np.ascontiguousarray(lo)


def prepare_inputs(hidden_states, positions, w_qkv, w_o):
    """Host-side sharding/preprocessing -> list of per-core input maps."""
    hidden_states = np.asarray(hidden_states, dtype=np.float32)
    positions = np.asarray(positions)
    w_qkv = np.asarray(w_qkv, dtype=np.float32)
    w_o = np.asarray(w_o, dtype=np.float32)

    # hidden^T -> [128, NCH, KT, CH] chunk-contiguous fp8 hi/lo
    hidT = (hidden_states.T.reshape(KT, 128, NCH, CH)
            .transpose(1, 2, 0, 3))
    hid_hi, hid_lo = _split8(hidT)

    pos = positions.astype(np.float32)
    half = HD // 2
    inv_freq = 1.0 / (10000.0 ** (np.arange(half, dtype=np.float32) / half))
    freqs = np.outer(pos, inv_freq)          # [T, 64]
    cos = np.cos(freqs).T                    # [64, T]
    sin = np.sin(freqs).T
    cos2 = np.concatenate([cos, cos], axis=0).astype(np.float16)
    sin2 = np.concatenate([-sin, sin], axis=0).astype(np.float16)

    j_idx = np.arange(128)[:, None]
    k_idx = np.arange(128)[None, :]
    ltri_np = np.where(j_idx < k_idx, MASK_NEG, 0.0).astype(ml_dtypes.bfloat16)
    eye_np = np.zeros((128, CH), dtype=ml_dtypes.bfloat16)
    eye_np[:, :128] = np.eye(128)

    in_maps = []
    for core in range(NCORES):
        heads = [HPC * core + i for i in range(HPC)]
        wq = [w_qkv[:, h * HD:(h + 1) * HD] for h in heads]
        wk = [w_qkv[:, FEAT * NCORES + h * HD:FEAT * NCORES + (h + 1) * HD]
              for h in heads]
        wv = [w_qkv[:, 2 * FEAT * NCORES + h * HD:2 * FEAT * NCORES + (h + 1) * HD]
              for h in heads]
        wqkv_core = np.concatenate(wq + wk + wv, axis=1) * WSCALE
        wqkv_core = wqkv_core.reshape(KT, 128, QKVF).transpose(1, 0, 2)
        wq_hi, wq_lo = _split8(wqkv_core)
        wq_hi_a = np.ascontiguousarray(wq_hi[:, :, :4 * HD])
        wq_hi_b = np.ascontiguousarray(wq_hi[:, :, 4 * HD:])
        wq_lo_a = np.ascontiguousarray(wq_lo[:, :, :4 * HD])
        wq_lo_b = np.ascontiguousarray(wq_lo[:, :, 4 * HD:])
        wo_core = np.stack(
            [w_o[h * HD:(h + 1) * HD, :] for h in heads], axis=0) * WSCALE
        wo_core = wo_core.transpose(1, 0, 2)   # [128, HPC, HID]
        wo_hi, wo_lo = _split8(wo_core)
        in_maps.append({
            "hid_hi": hid_hi,
            "hid_lo": hid_lo,
            "wq_hi_a": wq_hi_a,
            "wq_lo_a": wq_lo_a,
            "wq_hi_b": wq_hi_b,
            "wq_lo_b": wq_lo_b,
            "wo_hi": wo_hi,
            "wo_lo": wo_lo,
            "cos2": cos2,
            "sin2": sin2,
            "ltri": ltri_np,
            "eyepad": eye_np,
        })
    return in_maps


def kernel(hidden_states, positions, w_qkv, w_o):
    in_maps = prepare_inputs(hidden_states, positions, w_qkv, w_o)
    nc = _get_nc()
    try:
        res = run_bass_kernel_spmd(nc, in_maps, core_ids=list(range(NCORES)))
    except Exception:
        # transient device wedge from a prior crashed process: retry once
        res = run_bass_kernel_spmd(nc, in_maps, core_ids=list(range(NCORES)))
    acc = res.results[0]["out"].astype(np.float32)
    for i in range(1, NCORES):
        acc += res.results[i]["out"].astype(np.float32)
    return acc * (1.0 / (ATSCALE * WSCALE))
